# revision 1
# baseline (speedup 1.0000x reference)
"""Causal self-attention kernel for 8 TRN2 NeuronCores.

Problem (hardcoded): B=2, T=4096, C=768, NH=12, HS=64.
  qkv = x @ w_attn + b_attn; per-head causal softmax attention;
  y = att_out @ w_proj + b_proj

Sharding: 24 (batch, head) units over 8 cores -> 3 heads per core.
  cores 0..3: batch 0, heads (0,1,2), (3,4,5), (6,7,8), (9,10,11)
  cores 4..7: batch 1, same head split.
Each core computes a partial y^T [C, T]; the host sums partials per batch
and adds b_proj. The host also pre-transposes x (x^T is pure input
marshalling), so the kernel loads x^T chunks directly.

Per-core dataflow (f32r matmuls, fp32 PSUM accumulation), interleaved so
the PE never idles across phase boundaries (keeps the HAM clock warm):
  for tb in 0..7:
    QKV^T(tb) = w_blocks.T @ x^T(tb): Q^T/K^T [head_dim, T] layout; head C's
      Q/K are written twice ([qC|qC] weight blocks) so its S matmuls can
      alternate PE row groups like A/B do. V'^T blocks are 66 rows (64 v +
      ones row from a zero weight column with bias 1.0 + zero pad row),
      PE-transposed per 128-token tile into V' [128, 66].
    attention(qb=tb): per k-tile: S^T_A and S^T_B computed into one
      [128, 1024] PSUM pair with ALTERNATING PE row groups (A rows 0-63,
      B rows 64-127 -> the hardware overlaps them); one wide exp on ScalarE
      (scale=1/8) PSUM->SBUF; causal mask on diagonal tiles (DVE);
      O'^T += V'.T @ P^T accumulated in PSUM [66, 512] (row 64 = softmax
      denominator); then reciprocal + PE broadcast + DVE normalize.
      Head C runs the same way using its duplicated-row Q/K tiles
      (even k-tiles on rows 0-63, odd on rows 64-127).
    projection(qb=tb): y^T = sum_h wp_h.T @ O_norm_h.
"""

import numpy as np

B, T, C, NH = 2, 4096, 768, 12
HS = C // NH          # 64
NCORES = 8
HPC = 3               # heads per core
QB = 512              # q block (moving dim)
NQB = T // QB         # 8
NKT = T // 128        # 32 k-tiles
NTB = T // QB         # t-blocks
NCC = C // 128        # 6 contraction chunks
VP_W = 128            # V'' width per k-tile (64 v + ones + zero pad, FWL)
WQJ = 7 * 128   # 896

_CACHE = {}


def _build():
    import contextlib
    import concourse.bacc as bacc
    import concourse.mybir as mybir
    from concourse.tile import TileContext
    from concourse.masks import make_identity

    f32 = mybir.dt.float32
    f32r = mybir.dt.float32r
    bf16 = mybir.dt.bfloat16
    Exp = mybir.ActivationFunctionType.Exp
    mult = mybir.AluOpType.mult

    nc = bacc.Bacc(trn_type="TRN2")

    xt_d = nc.dram_tensor("xt", [C, T], f32, kind="ExternalInput")
    wq = nc.dram_tensor("wq", [C, WQJ], f32, kind="ExternalInput")
    bq = nc.dram_tensor("bq", [128, 7], f32, kind="ExternalInput")
    wp = nc.dram_tensor("wp", [192, C], f32, kind="ExternalInput")
    y = nc.dram_tensor("y", [C, T], f32, kind="ExternalOutput")

    # j-blocks: 0:[qA|qB] 1:[kA|kB] 2:[qC|qC] 3:[kC|kC] 4:vA' 5:vB' 6:vC'
    JBLK = [(0, 128), (128, 128), (256, 128), (384, 128),
            (512, 128), (640, 128), (768, 128)]

    with TileContext(nc) as tc, nc.allow_low_precision("f32r kernel"):
        with contextlib.ExitStack() as ctx:
            cpool = ctx.enter_context(tc.tile_pool(name="const", bufs=1))
            keep = ctx.enter_context(tc.tile_pool(name="keep", bufs=1))
            xtp_p = ctx.enter_context(tc.tile_pool(name="xtp", bufs=2))
            stg_p = ctx.enter_context(tc.tile_pool(name="stg", bufs=2))
            pt_p = ctx.enter_context(tc.tile_pool(name="ptp", bufs=3))
            on_p = ctx.enter_context(tc.tile_pool(name="onp", bufs=2))
            rr_p = ctx.enter_context(tc.tile_pool(name="rrp", bufs=2))
            ys_p = ctx.enter_context(tc.tile_pool(name="ysp", bufs=2))
            sps_p = ctx.enter_context(
                tc.tile_pool(name="sps", bufs=2, space="PSUM"))
            ov_p = ctx.enter_context(
                tc.tile_pool(name="ovp", bufs=1, space="PSUM"))
            sm_p = ctx.enter_context(
                tc.tile_pool(name="smp", bufs=2, space="PSUM"))

            ident_f = cpool.tile([128, 128], f32)
            make_identity(nc, ident_f[:])
            ident = cpool.tile([128, 128], f32r)
            nc.vector.tensor_copy(ident[:], ident_f[:])
            ident_b = cpool.tile([128, 128], bf16)
            nc.vector.tensor_copy(ident_b[:], ident_f[:])
            mask_b = cpool.tile([128, 896], bf16)
            mask = cpool.tile([128, 896], f32)
            nc.gpsimd.memset(mask[:], 1.0)
            nc.gpsimd.affine_select(
                out=mask[:], in_=mask[:], compare_op=mybir.AluOpType.is_ge,
                fill=0.0, base=-384, channel_multiplier=-1, pattern=[[1, 896]])
            ones_t = cpool.tile([128, 64], f32)
            nc.gpsimd.memset(ones_t[:], 1.0)
            nc.vector.tensor_copy(mask_b[:], mask[:])
            ones_r = cpool.tile([128, 64], f32r)
            nc.vector.tensor_copy(ones_r[:], ones_t[:])

            wq_sb = cpool.tile([128, NCC, WQJ], f32r)
            nc.gpsimd.dma_start(wq_sb[:],
                                wq.rearrange("(cc p) j -> p cc j", p=128))
            bq_sb = cpool.tile([128, 7], f32)
            nc.sync.dma_start(bq_sb[:], bq[:, :])
            wp_sb = [keep.tile([64, C], f32r, tag=f"wp{h}", name=f"wp{h}")
                     for h in range(HPC)]
            for h in range(HPC):
                nc.gpsimd.dma_start(wp_sb[h][:], wp[h * 64:(h + 1) * 64, :])

            QT_AB = keep.tile([128, T], bf16, tag="qt_ab")
            KT_AB = keep.tile([128, T], bf16, tag="kt_ab")
            QT_C = keep.tile([128, T], bf16, tag="qt_c")
            KT_C = keep.tile([128, T], bf16, tag="kt_c")
            Vp = [keep.tile([128, NKT * VP_W], bf16, tag=f"vp{h}",
                            name=f"vp{h}") for h in range(HPC)]

            def qkv_block(tb):
                t0 = tb * QB
                xt = xtp_p.tile([128, NCC, QB], f32r, tag="xt")
                for cc in range(NCC):
                    nc.gpsimd.dma_start(
                        xt[:, cc, :],
                        xt_d[cc * 128:(cc + 1) * 128, t0:t0 + QB])
                stage = [None] * 7
                for blk in range(7):
                    j0, m = JBLK[blk]
                    qp = sm_p.tile([128, QB], f32, tag="small",
                                   name=f"qp{tb}_{blk}")
                    for cc in range(NCC):
                        nc.tensor.matmul(
                            qp[0:m, :], wq_sb[:, cc, j0:j0 + m], xt[:, cc, :],
                            start=(cc == 0), stop=(cc == NCC - 1))
                    if blk == 0:
                        dest = QT_AB[:, t0:t0 + QB]
                    elif blk == 1:
                        dest = KT_AB[:, t0:t0 + QB]
                    elif blk == 2:
                        dest = QT_C[:, t0:t0 + QB]
                    elif blk == 3:
                        dest = KT_C[:, t0:t0 + QB]
                    else:
                        stage[blk] = stg_p.tile([128, QB], bf16,
                                                tag=f"stage{blk}",
                                                name=f"stage{tb}_{blk}")
                        dest = stage[blk][:]
                    nc.vector.tensor_scalar_add(
                        dest, qp[0:m, :], bq_sb[0:m, blk:blk + 1])
                for h in range(HPC):
                    src = stage[4 + h]
                    vtp = sm_p.tile([128, 4, VP_W], bf16, tag="small",
                                    name=f"vtp{tb}_{h}")
                    for i in range(4):
                        nc.tensor.transpose(
                            vtp[:, i, :], src[:, i * 128:(i + 1) * 128],
                            ident_b[:])
                    kt0 = tb * 4
                    vview = Vp[h][:].rearrange("p (kt w) -> p kt w", w=VP_W)
                    nc.vector.tensor_copy(vview[:, kt0:kt0 + 4, :], vtp[:])

            def s_pair(qb, sps, half, kt, kt_t, qt_t, rows):
                """S^T for one head-half into sps[:, half*QB:...]."""
                q0 = qb * QB
                r0, r1 = rows
                nc.tensor.matmul(
                    sps[:, half * QB:(half + 1) * QB],
                    kt_t[r0:r1, kt * 128:(kt + 1) * 128],
                    qt_t[r0:r1, q0:q0 + QB], start=True, stop=True)

            def mask_and_av(qb, sps_pt, half, kt, ov, start, stop, h):
                q0 = qb * QB
                m = kt * 128 - q0
                if 0 <= m < QB:
                    nc.vector.tensor_tensor(
                        out=sps_pt[:, half * QB:(half + 1) * QB],
                        in0=sps_pt[:, half * QB:(half + 1) * QB],
                        in1=mask_b[:, 384 - m:896 - m], op=mult)
                nc.tensor.matmul(
                    ov[:], Vp[h][:, kt * VP_W:(kt + 1) * VP_W],
                    sps_pt[:, half * QB:(half + 1) * QB],
                    start=start, stop=stop)

            def normalize(qb, h, ov, dest):
                q0 = qb * QB
                rr = rr_p.tile([66, QB], f32r, tag="rr", name=f"rr{qb}_{h}")
                nc.vector.reciprocal(rr[64:65, :], ov[64:65, :])
                rbp = sm_p.tile([64, QB], f32, tag="small", name=f"rb{qb}_{h}")
                nc.tensor.matmul(rbp[:], ones_r[64:65, :], rr[64:65, :],
                                 start=True, stop=True)
                rrb = rr_p.tile([64, QB], f32, tag="rrb", name=f"rc{qb}_{h}")
                nc.vector.tensor_copy(rrb[:], rbp[:])
                nc.vector.tensor_tensor(out=dest, in0=ov[0:64, :],
                                        in1=rrb[:], op=mult)

            qkv_block(0)
            for tb in range(NTB):
                qb = tb
                q0 = qb * QB
                nkt = 4 * qb + 4

                # heads A, B: row-group-alternated S, shared exp
                ovA = ov_p.tile([128, QB], f32, tag="ovA", name=f"ovA{qb}")
                ovB = ov_p.tile([128, QB], f32, tag="ovB", name=f"ovB{qb}")
                for kt in range(nkt):
                    sps = sps_p.tile([128, 1024], f32, tag="sps",
                                     name=f"sAB{qb}_{kt}")
                    s_pair(qb, sps, 0, kt, KT_AB, QT_AB, (0, 64))
                    s_pair(qb, sps, 1, kt, KT_AB, QT_AB, (64, 128))
                    pt = pt_p.tile([128, 1024], bf16, tag="pt")
                    nc.scalar.activation(pt[:], sps[:], Exp, scale=0.125)
                    mask_and_av(qb, pt, 0, kt, ovA, kt == 0, kt == nkt - 1, 0)
                    mask_and_av(qb, pt, 1, kt, ovB, kt == 0, kt == nkt - 1, 1)
                onA = on_p.tile([64, QB], f32r, tag="on0", name=f"onA{qb}")
                onB = on_p.tile([64, QB], f32r, tag="on1", name=f"onB{qb}")
                normalize(qb, 0, ovA, onA[:])
                normalize(qb, 1, ovB, onB[:])

                # head C: alternation via duplicated rows (even kt low,
                # odd kt high)
                ovC = ov_p.tile([128, QB], f32, tag="ovA", name=f"ovC{qb}")
                for s in range(nkt // 2):
                    kt0, kt1 = 2 * s, 2 * s + 1
                    sps = sps_p.tile([128, 1024], f32, tag="sps",
                                     name=f"sC{qb}_{s}")
                    s_pair(qb, sps, 0, kt0, KT_C, QT_C, (0, 64))
                    s_pair(qb, sps, 1, kt1, KT_C, QT_C, (64, 128))
                    pt = pt_p.tile([128, 1024], bf16, tag="pt")
                    nc.scalar.activation(pt[:], sps[:], Exp, scale=0.125)
                    mask_and_av(qb, pt, 0, kt0, ovC, s == 0, False, 2)
                    mask_and_av(qb, pt, 1, kt1, ovC, False,
                                s == nkt // 2 - 1, 2)
                onC = on_p.tile([64, QB], f32r, tag="on2", name=f"onC{qb}")
                normalize(qb, 2, ovC, onC[:])

                # projection for this q-block
                ons = [onA, onB, onC]
                for co in range(NCC):
                    yp = sm_p.tile([128, QB], f32, tag="small",
                                   name=f"yp{qb}_{co}")
                    for h in range(HPC):
                        nc.tensor.matmul(
                            yp[:], wp_sb[h][:, co * 128:(co + 1) * 128],
                            ons[h][:], start=(h == 0), stop=(h == HPC - 1))
                    ys = ys_p.tile([128, QB], f32, tag="ys",
                                   name=f"ys{qb}_{co}")
                    nc.vector.tensor_copy(ys[:], yp[:])
                    nc.sync.dma_start(
                        y[co * 128:(co + 1) * 128, q0:q0 + QB], ys[:])
                if tb + 1 < NTB:
                    qkv_block(tb + 1)

    nc.finalize()
    return nc


def _core_inputs(x, w_attn, b_attn, w_proj):
    """Build the 8 per-core input maps (numpy float32)."""
    maps = []
    zc = np.zeros((C, 64), np.float32)
    for core in range(NCORES):
        b = core // 4
        heads = [HPC * (core % 4) + k for k in range(HPC)]
        hA, hB, hC = heads
        qc = lambda h: slice(h * HS, (h + 1) * HS)
        kc = lambda h: slice(C + h * HS, C + (h + 1) * HS)
        vc = lambda h: slice(2 * C + h * HS, 2 * C + (h + 1) * HS)
        wqm = np.concatenate([
            w_attn[:, qc(hA)], w_attn[:, qc(hB)],
            w_attn[:, kc(hA)], w_attn[:, kc(hB)],
            w_attn[:, qc(hC)], w_attn[:, qc(hC)],
            w_attn[:, kc(hC)], w_attn[:, kc(hC)],
            w_attn[:, vc(hA)], zc, w_attn[:, vc(hB)], zc,
            w_attn[:, vc(hC)], zc,
        ], axis=1)
        bqm = np.zeros((128, 7), np.float32)
        bqm[0:64, 0] = b_attn[qc(hA)]
        bqm[64:128, 0] = b_attn[qc(hB)]
        bqm[0:64, 1] = b_attn[kc(hA)]
        bqm[64:128, 1] = b_attn[kc(hB)]
        bqm[0:64, 2] = b_attn[qc(hC)]
        bqm[64:128, 2] = b_attn[qc(hC)]
        bqm[0:64, 3] = b_attn[kc(hC)]
        bqm[64:128, 3] = b_attn[kc(hC)]
        for i, h in enumerate(heads):
            bqm[0:64, 4 + i] = b_attn[vc(h)]
            bqm[64, 4 + i] = 1.0
        wpm = np.concatenate([w_proj[h * HS:(h + 1) * HS, :] for h in heads],
                             axis=0)
        maps.append({
            "xt": np.ascontiguousarray(x[b].T, np.float32),
            "wq": np.ascontiguousarray(wqm, np.float32),
            "bq": np.ascontiguousarray(bqm, np.float32),
            "wp": np.ascontiguousarray(wpm, np.float32),
        })
    return maps


def run_cores(in_maps, trace=False):
    from concourse import bass_utils
    if "nc" not in _CACHE:
        _CACHE["nc"] = _build()
    return bass_utils.run_bass_kernel_spmd(
        _CACHE["nc"], in_maps, list(range(NCORES)), trace=trace)


def kernel(x, w_attn, b_attn, w_proj, b_proj):
    x = np.asarray(x, np.float32)
    w_attn = np.asarray(w_attn, np.float32)
    b_attn = np.asarray(b_attn, np.float32)
    w_proj = np.asarray(w_proj, np.float32)
    b_proj = np.asarray(b_proj, np.float32)

    in_maps = _core_inputs(x, w_attn, b_attn, w_proj)
    res = run_cores(in_maps)
    y = np.zeros((B, T, C), np.float32)
    for b in range(B):
        acc = np.zeros((C, T), np.float64)
        for core in range(4 * b, 4 * b + 4):
            acc += res.results[core]["y"].astype(np.float64)
        y[b] = acc.T + b_proj[None, :]
    return y



# revision 3
# speedup vs baseline: 1.1430x; 1.1430x over previous
"""Causal self-attention kernel for 8 TRN2 NeuronCores (v2).

Problem (hardcoded): B=2, T=4096, C=768, NH=12, HS=64.
  qkv = x @ w_attn + b_attn; per-head causal softmax attention;
  y = att_out @ w_proj + b_proj

Sharding: 24 (batch, head) units over 8 cores -> 3 heads per core.
  cores 0..3: batch 0, heads (0,1,2), (3,4,5), (6,7,8), (9,10,11)
  cores 4..7: batch 1, same head split.
Each core computes a partial y^T [C, T]; the host sums partials per batch
and adds b_eff = b_proj + sum_h bv_h @ w_proj[h] (the V-bias contribution
commutes with the softmax average exactly, so it folds into a host-side
constant). The host pre-transposes x (pure input marshalling).

v2 design notes (vs v1 baseline at ~560us):
  - ScalarE exp is the long pole (~248us of ACTIVATE); the whole schedule
    aims to keep ACT saturated and fill PE slack with QKV/proj matmuls.
  - V is computed directly in [token, dim] layout (stationary = x^T chunk,
    moving = wv) so the 96 PE transposes of v1 are gone; transpose-mode
    does not count as PE-busy for the HAM clock gate and was a main cause
    of the 14 observed K=4/8 cold oscillations (~220us at half clock).
  - Normalize path: reciprocal_approx_fast (DVE, ~5x faster than the 3.3us
    iterative reciprocal) + gpsimd partition_broadcast + one DVE multiply.
    v1 spent 80us (recip) + 60us (copies) of DVE here.
  - Projection packs heads A,B into one 128-contraction matmul.
  - Head C: its Q/K are written twice ([qC|qC], [kC|kC] weight blocks) so
    S_C can alternate PE row groups (even kt rows 0-63, odd kt rows 64-127)
    and pair up like A/B do.

Per-core dataflow per 512-token block tb (f32r QKV, bf16 attention):
  qkv_block(tb): Q^T/K^T blocks [128, 512] = wq.T @ x^T chunks;
    V tiles [128 tok, 256] = x @ wv (stationary x^T chunk), copied into
    V3 [128, kt, 3, 65] with a persistent ones-column at [..., 64].
  attention(qb=tb): per k-tile: S^T_A, S^T_B into one [128, 1024] PSUM
    pair on alternating row groups; one wide exp (ScalarE, scale=1/8);
    causal mask on diagonal tiles (DVE); AV accumulated per head into
    ov [65, 512] PSUM (row 64 = softmax denominator via the ones column).
    normalize: approx-recip of row 64 -> gpsimd broadcast -> DVE multiply.
  projection(qb=tb): y^T co-chunk = wpAB.T @ onAB + wpC.T @ onC.
"""

import numpy as np

B, T, C, NH = 2, 4096, 768, 12
HS = C // NH          # 64
NCORES = 8
HPC = 3               # heads per core
QB = 512              # q block (moving dim)
NQB = T // QB         # 8
NKT = T // 128        # 32 k-tiles
NTB = T // QB         # t-blocks
NCC = C // 128        # 6 contraction chunks
VP_W = 65             # per-head V block: 64 dims + ones column
WQJ = 4 * 128         # q/k weight blocks: [qA|qB | kA|kB | qC|qC | kC|kC]
WVJ = 256             # v moving width (192 used + 64 zero pad for f32r>=256)

_CACHE = {}


def _build():
    import contextlib
    import concourse.bacc as bacc
    import concourse.mybir as mybir
    from concourse.tile import TileContext

    f32 = mybir.dt.float32
    f32r = mybir.dt.float32r
    bf16 = mybir.dt.bfloat16
    Exp = mybir.ActivationFunctionType.Exp
    mult = mybir.AluOpType.mult

    nc = bacc.Bacc(trn_type="TRN2")

    xt_d = nc.dram_tensor("xt", [C, T], f32, kind="ExternalInput")
    wq = nc.dram_tensor("wq", [C, WQJ], f32, kind="ExternalInput")
    wv = nc.dram_tensor("wv", [C, WVJ], f32, kind="ExternalInput")
    bq = nc.dram_tensor("bq", [128, 4], f32, kind="ExternalInput")
    wpab = nc.dram_tensor("wpab", [128, C], f32, kind="ExternalInput")
    wpc = nc.dram_tensor("wpc", [64, C], f32, kind="ExternalInput")
    y = nc.dram_tensor("y", [C, T], f32, kind="ExternalOutput")

    with TileContext(nc) as tc, nc.allow_low_precision("f32r kernel"):
        with contextlib.ExitStack() as ctx:
            cpool = ctx.enter_context(tc.tile_pool(name="const", bufs=1))
            keep = ctx.enter_context(tc.tile_pool(name="keep", bufs=1))
            xtp_p = ctx.enter_context(tc.tile_pool(name="xtp", bufs=2))
            pt_p = ctx.enter_context(tc.tile_pool(name="ptp", bufs=3))
            on_p = ctx.enter_context(tc.tile_pool(name="onp", bufs=2))
            rr_p = ctx.enter_context(tc.tile_pool(name="rrp", bufs=2))
            ys_p = ctx.enter_context(tc.tile_pool(name="ysp", bufs=2))
            sps_p = ctx.enter_context(
                tc.tile_pool(name="sps", bufs=2, space="PSUM"))
            ov_p = ctx.enter_context(
                tc.tile_pool(name="ovp", bufs=1, space="PSUM"))
            sm_p = ctx.enter_context(
                tc.tile_pool(name="smp", bufs=2, space="PSUM"))

            # causal mask for diagonal tiles: mask[p, j] = 1 if j-384 <= ... ;
            # sliced as mask_b[:, 384-m:896-m] gives [128, 512] with
            # mask[k, q] = (k + m <= q) i.e. k_global <= q_global.
            mask = cpool.tile([128, 896], f32)
            nc.gpsimd.memset(mask[:], 1.0)
            nc.gpsimd.affine_select(
                out=mask[:], in_=mask[:], compare_op=mybir.AluOpType.is_ge,
                fill=0.0, base=-384, channel_multiplier=-1, pattern=[[1, 896]])
            mask_b = cpool.tile([128, 896], bf16)
            nc.vector.tensor_copy(mask_b[:], mask[:])

            wq_sb = cpool.tile([128, NCC, WQJ], f32r)
            nc.gpsimd.dma_start(wq_sb[:],
                                wq.rearrange("(cc p) j -> p cc j", p=128))
            wv_sb = cpool.tile([128, NCC, WVJ], f32r)
            nc.gpsimd.dma_start(wv_sb[:],
                                wv.rearrange("(cc p) j -> p cc j", p=128))
            bq_sb = cpool.tile([128, 4], f32)
            nc.sync.dma_start(bq_sb[:], bq[:, :])
            wpab_sb = cpool.tile([128, C], f32r)
            nc.gpsimd.dma_start(wpab_sb[:], wpab[:, :])
            wpc_sb = cpool.tile([64, C], f32r)
            nc.gpsimd.dma_start(wpc_sb[:], wpc[:, :])

            QT_AB = keep.tile([128, T], bf16, tag="qt_ab")
            KT_AB = keep.tile([128, T], bf16, tag="kt_ab")
            QT_C = keep.tile([128, T], bf16, tag="qt_c")
            KT_C = keep.tile([128, T], bf16, tag="kt_c")
            V3 = keep.tile([128, NKT, HPC, VP_W], bf16, tag="v3")
            # persistent ones column at [..., 64]; V copies only write 0:64
            nc.gpsimd.memset(V3[:, :, :, 64:65], 1.0)

            QK_DEST = [QT_AB, KT_AB, QT_C, KT_C]

            def qkv_block(tb):
                t0 = tb * QB
                xt = xtp_p.tile([128, NCC, QB], f32r, tag="xt")
                for cc in range(NCC):
                    nc.gpsimd.dma_start(
                        xt[:, cc, :],
                        xt_d[cc * 128:(cc + 1) * 128, t0:t0 + QB])
                for blk in range(4):
                    qp = sm_p.tile([128, QB], f32, tag="small",
                                   name=f"qp{tb}_{blk}")
                    for cc in range(NCC):
                        nc.tensor.matmul(
                            qp[:], wq_sb[:, cc, blk * 128:(blk + 1) * 128],
                            xt[:, cc, :],
                            start=(cc == 0), stop=(cc == NCC - 1))
                    nc.vector.tensor_scalar_add(
                        QK_DEST[blk][:, t0:t0 + QB], qp[:],
                        bq_sb[:, blk:blk + 1])
                for i in range(4):
                    vp = sm_p.tile([128, WVJ], f32, tag="small",
                                   name=f"vp{tb}_{i}")
                    for cc in range(NCC):
                        nc.tensor.matmul(
                            vp[:], xt[:, cc, i * 128:(i + 1) * 128],
                            wv_sb[:, cc, :],
                            start=(cc == 0), stop=(cc == NCC - 1))
                    dst = V3[:, tb * 4 + i, :, 0:64]
                    src = vp[:, 0:192].rearrange("p (h d) -> p h d", h=HPC)
                    nc.vector.tensor_copy(dst, src)

            def normalize(qb, ov, dest, nm):
                # custom-DVE ops mis-read PSUM on HW: stage den via SBUF
                dsb = rr_p.tile([1, QB], f32, tag="dsb", name=f"ds{nm}")
                nc.vector.tensor_copy(dsb[:], ov[64:65, :])
                rr = rr_p.tile([1, QB], f32, tag="rr", name=f"rr{nm}")
                nc.vector.reciprocal_approx_fast(rr[:], dsb[:])
                rrb = rr_p.tile([64, QB], f32, tag="rrb", name=f"rrb{nm}")
                nc.gpsimd.partition_broadcast(rrb[:], rr[:])
                nc.vector.tensor_tensor(out=dest, in0=ov[0:64, :],
                                        in1=rrb[:], op=mult)

            qkv_block(0)
            for tb in range(NTB):
                qb = tb
                q0 = qb * QB
                nkt = 4 * qb + 4

                # heads A, B: row-group-alternated S, shared exp
                ovA = ov_p.tile([65, QB], f32, tag="ovA", name=f"ovA{qb}")
                ovB = ov_p.tile([65, QB], f32, tag="ovB", name=f"ovB{qb}")
                for kt in range(nkt):
                    k0 = kt * 128
                    sps = sps_p.tile([128, 1024], f32, tag="sps",
                                     name=f"sAB{qb}_{kt}")
                    nc.tensor.matmul(
                        sps[:, 0:QB], KT_AB[0:64, k0:k0 + 128],
                        QT_AB[0:64, q0:q0 + QB], start=True, stop=True)
                    nc.tensor.matmul(
                        sps[:, QB:2 * QB], KT_AB[64:128, k0:k0 + 128],
                        QT_AB[64:128, q0:q0 + QB], start=True, stop=True)
                    pt = pt_p.tile([128, 1024], bf16, tag="pt")
                    nc.scalar.activation(pt[:], sps[:], Exp, scale=0.125)
                    m = k0 - q0
                    if 0 <= m < QB:
                        for h in range(2):
                            nc.vector.tensor_tensor(
                                out=pt[:, h * QB:(h + 1) * QB],
                                in0=pt[:, h * QB:(h + 1) * QB],
                                in1=mask_b[:, 384 - m:896 - m], op=mult)
                    nc.tensor.matmul(ovA[:], V3[:, kt, 0, :], pt[:, 0:QB],
                                     start=(kt == 0), stop=(kt == nkt - 1))
                    nc.tensor.matmul(ovB[:], V3[:, kt, 1, :],
                                     pt[:, QB:2 * QB],
                                     start=(kt == 0), stop=(kt == nkt - 1))
                onAB = on_p.tile([128, QB], f32r, tag="onAB",
                                 name=f"onAB{qb}")
                normalize(qb, ovA, onAB[0:64, :], f"A{qb}")
                normalize(qb, ovB, onAB[64:128, :], f"B{qb}")

                # head C: row alternation via duplicated Q/K rows
                # (even kt on rows 0-63, odd kt on rows 64-127)
                ovC = ov_p.tile([65, QB], f32, tag="ovA", name=f"ovC{qb}")
                for s in range(nkt // 2):
                    kt0, kt1 = 2 * s, 2 * s + 1
                    sps = sps_p.tile([128, 1024], f32, tag="sps",
                                     name=f"sC{qb}_{s}")
                    nc.tensor.matmul(
                        sps[:, 0:QB], KT_C[0:64, kt0 * 128:kt0 * 128 + 128],
                        QT_C[0:64, q0:q0 + QB], start=True, stop=True)
                    nc.tensor.matmul(
                        sps[:, QB:2 * QB],
                        KT_C[64:128, kt1 * 128:kt1 * 128 + 128],
                        QT_C[64:128, q0:q0 + QB], start=True, stop=True)
                    pt = pt_p.tile([128, 1024], bf16, tag="pt")
                    nc.scalar.activation(pt[:], sps[:], Exp, scale=0.125)
                    for half, kt in ((0, kt0), (1, kt1)):
                        m = kt * 128 - q0
                        if 0 <= m < QB:
                            nc.vector.tensor_tensor(
                                out=pt[:, half * QB:(half + 1) * QB],
                                in0=pt[:, half * QB:(half + 1) * QB],
                                in1=mask_b[:, 384 - m:896 - m], op=mult)
                    nc.tensor.matmul(ovC[:], V3[:, kt0, 2, :], pt[:, 0:QB],
                                     start=(s == 0), stop=False)
                    nc.tensor.matmul(ovC[:], V3[:, kt1, 2, :],
                                     pt[:, QB:2 * QB],
                                     start=False, stop=(s == nkt // 2 - 1))
                onC = on_p.tile([64, QB], f32r, tag="onC", name=f"onC{qb}")
                normalize(qb, ovC, onC[:], f"C{qb}")

                # projection for this q-block
                for co in range(NCC):
                    yp = sm_p.tile([128, QB], f32, tag="small",
                                   name=f"yp{qb}_{co}")
                    nc.tensor.matmul(
                        yp[:], wpab_sb[:, co * 128:(co + 1) * 128],
                        onAB[:], start=True, stop=False)
                    nc.tensor.matmul(
                        yp[:], wpc_sb[:, co * 128:(co + 1) * 128],
                        onC[:], start=False, stop=True)
                    ys = ys_p.tile([128, QB], f32, tag="ys",
                                   name=f"ys{qb}_{co}")
                    nc.vector.tensor_copy(ys[:], yp[:])
                    nc.sync.dma_start(
                        y[co * 128:(co + 1) * 128, q0:q0 + QB], ys[:])
                if tb + 1 < NTB:
                    qkv_block(tb + 1)

    nc.finalize()
    return nc


def _core_inputs(x, w_attn, b_attn, w_proj):
    """Build the 8 per-core input maps (numpy float32)."""
    maps = []
    for core in range(NCORES):
        b = core // 4
        heads = [HPC * (core % 4) + k for k in range(HPC)]
        hA, hB, hC = heads
        qc = lambda h: slice(h * HS, (h + 1) * HS)
        kc = lambda h: slice(C + h * HS, C + (h + 1) * HS)
        vc = lambda h: slice(2 * C + h * HS, 2 * C + (h + 1) * HS)
        wqm = np.concatenate([
            w_attn[:, qc(hA)], w_attn[:, qc(hB)],
            w_attn[:, kc(hA)], w_attn[:, kc(hB)],
            w_attn[:, qc(hC)], w_attn[:, qc(hC)],
            w_attn[:, kc(hC)], w_attn[:, kc(hC)],
        ], axis=1)
        wvm = np.concatenate(
            [w_attn[:, vc(h)] for h in heads] + [np.zeros((C, 64), np.float32)],
            axis=1)
        bqm = np.zeros((128, 4), np.float32)
        bqm[0:64, 0] = b_attn[qc(hA)]
        bqm[64:128, 0] = b_attn[qc(hB)]
        bqm[0:64, 1] = b_attn[kc(hA)]
        bqm[64:128, 1] = b_attn[kc(hB)]
        bqm[0:64, 2] = b_attn[qc(hC)]
        bqm[64:128, 2] = b_attn[qc(hC)]
        bqm[0:64, 3] = b_attn[kc(hC)]
        bqm[64:128, 3] = b_attn[kc(hC)]
        wpabm = np.concatenate([w_proj[hA * HS:(hA + 1) * HS, :],
                                w_proj[hB * HS:(hB + 1) * HS, :]], axis=0)
        wpcm = w_proj[hC * HS:(hC + 1) * HS, :]
        maps.append({
            "xt": np.ascontiguousarray(x[b].T, np.float32),
            "wq": np.ascontiguousarray(wqm, np.float32),
            "wv": np.ascontiguousarray(wvm, np.float32),
            "bq": np.ascontiguousarray(bqm, np.float32),
            "wpab": np.ascontiguousarray(wpabm, np.float32),
            "wpc": np.ascontiguousarray(wpcm, np.float32),
        })
    return maps


def run_cores(in_maps, trace=False):
    from concourse import bass_utils
    if "nc" not in _CACHE:
        _CACHE["nc"] = _build()
    return bass_utils.run_bass_kernel_spmd(
        _CACHE["nc"], in_maps, list(range(NCORES)), trace=trace)


def kernel(x, w_attn, b_attn, w_proj, b_proj):
    x = np.asarray(x, np.float32)
    w_attn = np.asarray(w_attn, np.float32)
    b_attn = np.asarray(b_attn, np.float32)
    w_proj = np.asarray(w_proj, np.float32)
    b_proj = np.asarray(b_proj, np.float32)

    # V-bias folds exactly into a constant row: sum_k P/den = 1, so
    # O_h = AV_h/den + bv_h and its projection adds bv_h @ W_h.
    b_eff = b_proj.astype(np.float64).copy()
    for h in range(NH):
        bv = b_attn[2 * C + h * HS:2 * C + (h + 1) * HS].astype(np.float64)
        b_eff += bv @ w_proj[h * HS:(h + 1) * HS, :].astype(np.float64)

    in_maps = _core_inputs(x, w_attn, b_attn, w_proj)
    res = run_cores(in_maps)
    y = np.zeros((B, T, C), np.float32)
    for b in range(B):
        acc = np.zeros((C, T), np.float64)
        for core in range(4 * b, 4 * b + 4):
            acc += res.results[core]["y"].astype(np.float64)
        y[b] = (acc.T + b_eff[None, :]).astype(np.float32)
    return y


# revision 5
# speedup vs baseline: 1.4332x; 1.2539x over previous
"""Causal self-attention kernel for 8 TRN2 NeuronCores (v3).

Problem (hardcoded): B=2, T=4096, C=768, NH=12, HS=64.
  qkv = x @ w_attn + b_attn; per-head causal softmax attention;
  y = att_out @ w_proj + b_proj

Sharding: 24 (batch, head) units over 8 cores -> 3 heads per core.
  cores 0..3: batch 0, heads (0,1,2), (3,4,5), (6,7,8), (9,10,11)
  cores 4..7: batch 1, same head split.
Each core computes a partial y^T [C, T]; the host sums partials per batch
and adds b_eff = b_proj + sum_h bv_h @ w_proj[h] (the V-bias contribution
commutes with the softmax average exactly: sum_k P/den = 1).

v3 structure (ScalarE exp is the hard floor at ~1 elem/cycle/lane;
everything else is arranged to keep ACT saturated and PE warm):
  - The QKV matmul groups for block tb+1 are emitted INTERLEAVED into the
    attention kt-loops of block tb, so the PE always has independent work
    during exp waits and ScalarE never drains at phase boundaries (v2
    showed ~22us ACT idle per tb and 14 HAM re-throttles without this).
  - Diagonal k-tiles are q-trimmed: S/exp/AV only touch q >= k_tile_base,
    the causal mask multiply shrinks to a constant [128,128] triangle.
  - V is computed directly in [token, dim] layout (stationary = x^T chunk,
    moving = wv) - no PE transposes (transpose-mode doesn't count as
    PE-busy for the HAM clock gate).
  - Normalize: den row staged to SBUF (custom-DVE ops mis-read PSUM on HW),
    reciprocal_approx_fast, gpsimd partition_broadcast, one DVE multiply.
  - Projection packs heads A,B into one 128-contraction matmul; head C's
    Q/K are duplicated ([qC|qC], [kC|kC]) so S_C pairs PE row groups.
"""

import numpy as np

B, T, C, NH = 2, 4096, 768, 12
HS = C // NH          # 64
NCORES = 8
HPC = 3               # heads per core
QB = 512              # q block (moving dim)
NQB = T // QB         # 8
NKT = T // 128        # 32 k-tiles
NTB = T // QB         # t-blocks
NCC = C // 128        # 6 contraction chunks
VP_W = 65             # per-head V block: 64 dims + ones column
WQJ = 4 * 128         # q/k weight blocks: [qA|qB | kA|kB | qC|qC | kC|kC]
WVJ = 256             # v moving width (192 used + 64 zero pad for f32r>=256)

_CACHE = {}


def _build():
    import contextlib
    import concourse.bacc as bacc
    import concourse.mybir as mybir
    from concourse.tile import TileContext

    f32 = mybir.dt.float32
    f32r = mybir.dt.float32r
    bf16 = mybir.dt.bfloat16
    Exp = mybir.ActivationFunctionType.Exp
    mult = mybir.AluOpType.mult

    nc = bacc.Bacc(trn_type="TRN2")

    xt_d = nc.dram_tensor("xt", [C, T], f32, kind="ExternalInput")
    wq = nc.dram_tensor("wq", [C, WQJ], f32, kind="ExternalInput")
    wv = nc.dram_tensor("wv", [C, WVJ], f32, kind="ExternalInput")
    bq = nc.dram_tensor("bq", [128, 4], f32, kind="ExternalInput")
    wpab = nc.dram_tensor("wpab", [128, C], f32, kind="ExternalInput")
    wpc = nc.dram_tensor("wpc", [64, C], f32, kind="ExternalInput")
    y = nc.dram_tensor("y", [C, T], f32, kind="ExternalOutput")

    with TileContext(nc) as tc, nc.allow_low_precision("f32r kernel"):
        with contextlib.ExitStack() as ctx:
            cpool = ctx.enter_context(tc.tile_pool(name="const", bufs=1))
            keep = ctx.enter_context(tc.tile_pool(name="keep", bufs=1))
            xtp_p = ctx.enter_context(tc.tile_pool(name="xtp", bufs=2))
            pt_p = ctx.enter_context(tc.tile_pool(name="ptp", bufs=4))
            on_p = ctx.enter_context(tc.tile_pool(name="onp", bufs=2))
            rr_p = ctx.enter_context(tc.tile_pool(name="rrp", bufs=2))
            ys_p = ctx.enter_context(tc.tile_pool(name="ysp", bufs=2))
            sps_p = ctx.enter_context(
                tc.tile_pool(name="sps", bufs=2, space="PSUM"))
            ov_p = ctx.enter_context(
                tc.tile_pool(name="ovp", bufs=1, space="PSUM"))
            sm_p = ctx.enter_context(
                tc.tile_pool(name="smp", bufs=2, space="PSUM"))

            # constant [128,128] lower-triangle mask: tri[k, q] = (k <= q)
            tri = cpool.tile([128, 128], f32)
            nc.gpsimd.memset(tri[:], 1.0)
            nc.gpsimd.affine_select(
                out=tri[:], in_=tri[:], compare_op=mybir.AluOpType.is_ge,
                fill=0.0, base=0, channel_multiplier=-1, pattern=[[1, 128]])
            tri_b = cpool.tile([128, 128], bf16)
            nc.vector.tensor_copy(tri_b[:], tri[:])

            wq_sb = cpool.tile([128, NCC, WQJ], f32r)
            nc.gpsimd.dma_start(wq_sb[:],
                                wq.rearrange("(cc p) j -> p cc j", p=128))
            wv_sb = cpool.tile([128, NCC, WVJ], f32r)
            nc.gpsimd.dma_start(wv_sb[:],
                                wv.rearrange("(cc p) j -> p cc j", p=128))
            bq_sb = cpool.tile([128, 4], f32)
            nc.sync.dma_start(bq_sb[:], bq[:, :])
            wpab_sb = cpool.tile([128, C], f32r)
            nc.gpsimd.dma_start(wpab_sb[:], wpab[:, :])
            wpc_sb = cpool.tile([64, C], f32r)
            nc.gpsimd.dma_start(wpc_sb[:], wpc[:, :])

            QT_AB = keep.tile([128, T], bf16, tag="qt_ab")
            KT_AB = keep.tile([128, T], bf16, tag="kt_ab")
            QT_C = keep.tile([128, T], bf16, tag="qt_c")
            KT_C = keep.tile([128, T], bf16, tag="kt_c")
            V3 = keep.tile([128, NKT, HPC, VP_W], bf16, tag="v3")
            # persistent ones column at [..., 64]; V copies only write 0:64
            nc.gpsimd.memset(V3[:, :, :, 64:65], 1.0)

            QK_DEST = [QT_AB, KT_AB, QT_C, KT_C]

            _xt = {}

            def qkv_dmas(tb):
                t0 = tb * QB
                xt = xtp_p.tile([128, NCC, QB], f32r, tag="xt",
                                name=f"xt{tb}")
                for cc in range(NCC):
                    nc.gpsimd.dma_start(
                        xt[:, cc, :],
                        xt_d[cc * 128:(cc + 1) * 128, t0:t0 + QB])
                _xt[tb] = xt

            def qkv_group(tb, g):
                """g in 0..3: Q/K weight block g; g in 4..7: V token tile."""
                t0 = tb * QB
                xt = _xt[tb]
                if g < 4:
                    qp = sm_p.tile([128, QB], f32, tag="small",
                                   name=f"qp{tb}_{g}")
                    for cc in range(NCC):
                        nc.tensor.matmul(
                            qp[:], wq_sb[:, cc, g * 128:(g + 1) * 128],
                            xt[:, cc, :],
                            start=(cc == 0), stop=(cc == NCC - 1))
                    nc.vector.tensor_scalar_add(
                        QK_DEST[g][:, t0:t0 + QB], qp[:], bq_sb[:, g:g + 1])
                else:
                    i = g - 4
                    vp = sm_p.tile([128, WVJ], f32, tag="small",
                                   name=f"vp{tb}_{i}")
                    for cc in range(NCC):
                        nc.tensor.matmul(
                            vp[:], xt[:, cc, i * 128:(i + 1) * 128],
                            wv_sb[:, cc, :],
                            start=(cc == 0), stop=(cc == NCC - 1))
                    dst = V3[:, tb * 4 + i, :, 0:64]
                    src = vp[:, 0:192].rearrange("p (h d) -> p h d", h=HPC)
                    nc.vector.tensor_copy(dst, src)

            def normalize(ov, dest, nm):
                # custom-DVE ops mis-read PSUM on HW: stage den via SBUF
                dsb = rr_p.tile([1, QB], f32, tag="dsb", name=f"ds{nm}")
                nc.vector.tensor_copy(dsb[:], ov[64:65, :])
                rr = rr_p.tile([1, QB], f32, tag="rr", name=f"rr{nm}")
                nc.vector.reciprocal_approx_fast(rr[:], dsb[:])
                rrb = rr_p.tile([64, QB], f32, tag="rrb", name=f"rrb{nm}")
                nc.gpsimd.partition_broadcast(rrb[:], rr[:])
                nc.vector.tensor_tensor(out=dest, in0=ov[0:64, :],
                                        in1=rrb[:], op=mult)

            qkv_dmas(0)
            for g in range(8):
                qkv_group(0, g)

            for tb in range(NTB):
                qb = tb
                q0 = qb * QB
                nkt = 4 * qb + 4

                # spread next block's QKV groups across this block's
                # attention iterations (AB loop: nkt, C loop: nkt//2)
                filler = {}
                if tb + 1 < NTB:
                    qkv_dmas(tb + 1)
                    total = nkt + nkt // 2
                    for g in range(8):
                        pos = min(total - 1, (g * total) // 8)
                        filler.setdefault(pos, []).append(g)

                def run_filler(pos):
                    for g in filler.get(pos, ()):
                        qkv_group(tb + 1, g)

                # heads A, B: row-group-alternated S, shared exp
                ovA = ov_p.tile([65, QB], f32, tag="ovA", name=f"ovA{qb}")
                ovB = ov_p.tile([65, QB], f32, tag="ovB", name=f"ovB{qb}")
                for kt in range(nkt):
                    k0 = kt * 128
                    m = k0 - q0
                    qo = max(0, m)  # q-trim on diagonal tiles
                    sps = sps_p.tile([128, 1024], f32, tag="sps",
                                     name=f"sAB{qb}_{kt}")
                    nc.tensor.matmul(
                        sps[:, qo:QB], KT_AB[0:64, k0:k0 + 128],
                        QT_AB[0:64, q0 + qo:q0 + QB], start=True, stop=True)
                    nc.tensor.matmul(
                        sps[:, QB + qo:2 * QB], KT_AB[64:128, k0:k0 + 128],
                        QT_AB[64:128, q0 + qo:q0 + QB],
                        start=True, stop=True)
                    pt = pt_p.tile([128, 2, QB], bf16, tag="pt")
                    spsv = sps[:].rearrange("p (h q) -> p h q", h=2)
                    nc.scalar.activation(pt[:, :, qo:QB], spsv[:, :, qo:QB],
                                         Exp, scale=0.125)
                    if 0 <= m < QB:
                        for h in range(2):
                            nc.vector.tensor_tensor(
                                out=pt[:, h, m:m + 128],
                                in0=pt[:, h, m:m + 128],
                                in1=tri_b[:], op=mult)
                    nc.tensor.matmul(ovA[:, qo:QB], V3[:, kt, 0, :],
                                     pt[:, 0, qo:QB],
                                     start=(kt == 0), stop=(kt == nkt - 1))
                    nc.tensor.matmul(ovB[:, qo:QB], V3[:, kt, 1, :],
                                     pt[:, 1, qo:QB],
                                     start=(kt == 0), stop=(kt == nkt - 1))
                    run_filler(kt)
                onAB = on_p.tile([128, QB], f32r, tag="onAB",
                                 name=f"onAB{qb}")
                normalize(ovA, onAB[0:64, :], f"A{qb}")
                normalize(ovB, onAB[64:128, :], f"B{qb}")

                # head C: row alternation via duplicated Q/K rows
                # (even kt on rows 0-63, odd kt on rows 64-127)
                ovC = ov_p.tile([65, QB], f32, tag="ovA", name=f"ovC{qb}")
                for s in range(nkt // 2):
                    kt0, kt1 = 2 * s, 2 * s + 1
                    m0 = kt0 * 128 - q0
                    m1 = m0 + 128
                    qo0, qo1 = max(0, m0), max(0, m1)
                    sps = sps_p.tile([128, 1024], f32, tag="sps",
                                     name=f"sC{qb}_{s}")
                    nc.tensor.matmul(
                        sps[:, qo0:QB],
                        KT_C[0:64, kt0 * 128:kt0 * 128 + 128],
                        QT_C[0:64, q0 + qo0:q0 + QB], start=True, stop=True)
                    # half1 writes the full shared range [qo0, QB) so the
                    # shared exp reads only written PSUM; AV reads [qo1, QB)
                    nc.tensor.matmul(
                        sps[:, QB + qo0:2 * QB],
                        KT_C[64:128, kt1 * 128:kt1 * 128 + 128],
                        QT_C[64:128, q0 + qo0:q0 + QB],
                        start=True, stop=True)
                    pt = pt_p.tile([128, 2, QB], bf16, tag="pt")
                    spsv = sps[:].rearrange("p (h q) -> p h q", h=2)
                    # shared exp at the wider range; half1's extra cols are
                    # stale-but-finite and never read by its trimmed AV
                    nc.scalar.activation(pt[:, :, qo0:QB],
                                         spsv[:, :, qo0:QB],
                                         Exp, scale=0.125)
                    for half, m in ((0, m0), (1, m1)):
                        if 0 <= m < QB:
                            nc.vector.tensor_tensor(
                                out=pt[:, half, m:m + 128],
                                in0=pt[:, half, m:m + 128],
                                in1=tri_b[:], op=mult)
                    nc.tensor.matmul(ovC[:, qo0:QB], V3[:, kt0, 2, :],
                                     pt[:, 0, qo0:QB],
                                     start=(s == 0), stop=False)
                    nc.tensor.matmul(ovC[:, qo1:QB], V3[:, kt1, 2, :],
                                     pt[:, 1, qo1:QB],
                                     start=False, stop=(s == nkt // 2 - 1))
                    run_filler(nkt + s)
                onC = on_p.tile([64, QB], f32r, tag="onC", name=f"onC{qb}")
                normalize(ovC, onC[:], f"C{qb}")

                # projection for this q-block
                for co in range(NCC):
                    yp = sm_p.tile([128, QB], f32, tag="small",
                                   name=f"yp{qb}_{co}")
                    nc.tensor.matmul(
                        yp[:], wpab_sb[:, co * 128:(co + 1) * 128],
                        onAB[:], start=True, stop=False)
                    nc.tensor.matmul(
                        yp[:], wpc_sb[:, co * 128:(co + 1) * 128],
                        onC[:], start=False, stop=True)
                    ys = ys_p.tile([128, QB], f32, tag="ys",
                                   name=f"ys{qb}_{co}")
                    nc.vector.tensor_copy(ys[:], yp[:])
                    nc.sync.dma_start(
                        y[co * 128:(co + 1) * 128, q0:q0 + QB], ys[:])

    nc.finalize()
    return nc


def _core_inputs(x, w_attn, b_attn, w_proj):
    """Build the 8 per-core input maps (numpy float32)."""
    maps = []
    for core in range(NCORES):
        b = core // 4
        heads = [HPC * (core % 4) + k for k in range(HPC)]
        hA, hB, hC = heads
        qc = lambda h: slice(h * HS, (h + 1) * HS)
        kc = lambda h: slice(C + h * HS, C + (h + 1) * HS)
        vc = lambda h: slice(2 * C + h * HS, 2 * C + (h + 1) * HS)
        wqm = np.concatenate([
            w_attn[:, qc(hA)], w_attn[:, qc(hB)],
            w_attn[:, kc(hA)], w_attn[:, kc(hB)],
            w_attn[:, qc(hC)], w_attn[:, qc(hC)],
            w_attn[:, kc(hC)], w_attn[:, kc(hC)],
        ], axis=1)
        wvm = np.concatenate(
            [w_attn[:, vc(h)] for h in heads] + [np.zeros((C, 64), np.float32)],
            axis=1)
        bqm = np.zeros((128, 4), np.float32)
        bqm[0:64, 0] = b_attn[qc(hA)]
        bqm[64:128, 0] = b_attn[qc(hB)]
        bqm[0:64, 1] = b_attn[kc(hA)]
        bqm[64:128, 1] = b_attn[kc(hB)]
        bqm[0:64, 2] = b_attn[qc(hC)]
        bqm[64:128, 2] = b_attn[qc(hC)]
        bqm[0:64, 3] = b_attn[kc(hC)]
        bqm[64:128, 3] = b_attn[kc(hC)]
        wpabm = np.concatenate([w_proj[hA * HS:(hA + 1) * HS, :],
                                w_proj[hB * HS:(hB + 1) * HS, :]], axis=0)
        wpcm = w_proj[hC * HS:(hC + 1) * HS, :]
        maps.append({
            "xt": np.ascontiguousarray(x[b].T, np.float32),
            "wq": np.ascontiguousarray(wqm, np.float32),
            "wv": np.ascontiguousarray(wvm, np.float32),
            "bq": np.ascontiguousarray(bqm, np.float32),
            "wpab": np.ascontiguousarray(wpabm, np.float32),
            "wpc": np.ascontiguousarray(wpcm, np.float32),
        })
    return maps


def run_cores(in_maps, trace=False):
    from concourse import bass_utils
    if "nc" not in _CACHE:
        _CACHE["nc"] = _build()
    return bass_utils.run_bass_kernel_spmd(
        _CACHE["nc"], in_maps, list(range(NCORES)), trace=trace)


def kernel(x, w_attn, b_attn, w_proj, b_proj):
    x = np.asarray(x, np.float32)
    w_attn = np.asarray(w_attn, np.float32)
    b_attn = np.asarray(b_attn, np.float32)
    w_proj = np.asarray(w_proj, np.float32)
    b_proj = np.asarray(b_proj, np.float32)

    # V-bias folds exactly into a constant row: sum_k P/den = 1, so
    # O_h = AV_h/den + bv_h and its projection adds bv_h @ W_h.
    b_eff = b_proj.astype(np.float64).copy()
    for h in range(NH):
        bv = b_attn[2 * C + h * HS:2 * C + (h + 1) * HS].astype(np.float64)
        b_eff += bv @ w_proj[h * HS:(h + 1) * HS, :].astype(np.float64)

    in_maps = _core_inputs(x, w_attn, b_attn, w_proj)
    res = run_cores(in_maps)
    y = np.zeros((B, T, C), np.float32)
    for b in range(B):
        acc = np.zeros((C, T), np.float64)
        for core in range(4 * b, 4 * b + 4):
            acc += res.results[core]["y"].astype(np.float64)
        y[b] = (acc.T + b_eff[None, :]).astype(np.float32)
    return y


# revision 9
# speedup vs baseline: 1.5133x; 1.0559x over previous
"""Causal self-attention kernel for 8 TRN2 NeuronCores (v3).

Problem (hardcoded): B=2, T=4096, C=768, NH=12, HS=64.
  qkv = x @ w_attn + b_attn; per-head causal softmax attention;
  y = att_out @ w_proj + b_proj

Sharding: 24 (batch, head) units over 8 cores -> 3 heads per core.
  cores 0..3: batch 0, heads (0,1,2), (3,4,5), (6,7,8), (9,10,11)
  cores 4..7: batch 1, same head split.
Each core computes a partial y^T [C, T]; the host sums partials per batch
and adds b_eff = b_proj + sum_h bv_h @ w_proj[h] (the V-bias contribution
commutes with the softmax average exactly: sum_k P/den = 1).

v3 structure (ScalarE exp is the hard floor at ~1 elem/cycle/lane;
everything else is arranged to keep ACT saturated and PE warm):
  - The QKV matmul groups for block tb+1 are emitted INTERLEAVED into the
    attention kt-loops of block tb, so the PE always has independent work
    during exp waits and ScalarE never drains at phase boundaries (v2
    showed ~22us ACT idle per tb and 14 HAM re-throttles without this).
  - Diagonal k-tiles are q-trimmed: S/exp/AV only touch q >= k_tile_base,
    the causal mask multiply shrinks to a constant [128,128] triangle.
  - V is computed directly in [token, dim] layout (stationary = x^T chunk,
    moving = wv) - no PE transposes (transpose-mode doesn't count as
    PE-busy for the HAM clock gate).
  - Normalize: den row staged to SBUF (custom-DVE ops mis-read PSUM on HW),
    reciprocal_approx_fast, gpsimd partition_broadcast, one DVE multiply.
  - Projection packs heads A,B into one 128-contraction matmul; head C's
    Q/K are duplicated ([qC|qC], [kC|kC]) so S_C pairs PE row groups.
"""

import numpy as np

B, T, C, NH = 2, 4096, 768, 12
HS = C // NH          # 64
NCORES = 8
HPC = 3               # heads per core
QB = 512              # q block (moving dim)
NQB = T // QB         # 8
NKT = T // 128        # 32 k-tiles
NTB = T // QB         # t-blocks
NCC = C // 128        # 6 contraction chunks
VP_W = 65             # per-head V block: 64 dims + ones column
WQJ = 4 * 128         # q/k weight blocks: [qA|qB | kA|kB | qC|qC | kC|kC]
WVJ = 256             # v moving width (192 used + 64 zero pad for f32r>=256)

_CACHE = {}


def _build():
    import contextlib
    import concourse.bacc as bacc
    import concourse.mybir as mybir
    from concourse.tile import TileContext

    f32 = mybir.dt.float32
    f32r = mybir.dt.float32r
    bf16 = mybir.dt.bfloat16
    Exp = mybir.ActivationFunctionType.Exp
    mult = mybir.AluOpType.mult

    nc = bacc.Bacc(trn_type="TRN2")

    xt_d = nc.dram_tensor("xt", [C, T], f32, kind="ExternalInput")
    wq = nc.dram_tensor("wq", [C, WQJ], f32, kind="ExternalInput")
    wv = nc.dram_tensor("wv", [C, WVJ], f32, kind="ExternalInput")
    bq = nc.dram_tensor("bq", [128, 4], f32, kind="ExternalInput")
    wpab = nc.dram_tensor("wpab", [128, C], f32, kind="ExternalInput")
    wpc = nc.dram_tensor("wpc", [64, C], f32, kind="ExternalInput")
    y = nc.dram_tensor("y", [C, T], f32, kind="ExternalOutput")

    with TileContext(nc) as tc, nc.allow_low_precision("f32r kernel"):
        with contextlib.ExitStack() as ctx:
            cpool = ctx.enter_context(tc.tile_pool(name="const", bufs=1))
            keep = ctx.enter_context(tc.tile_pool(name="keep", bufs=1))
            xtp_p = ctx.enter_context(tc.tile_pool(name="xtp", bufs=2))
            pt_p = ctx.enter_context(tc.tile_pool(name="ptp", bufs=4))
            on_p = ctx.enter_context(tc.tile_pool(name="onp", bufs=2))
            rr_p = ctx.enter_context(tc.tile_pool(name="rrp", bufs=2))
            ys_p = ctx.enter_context(tc.tile_pool(name="ysp", bufs=2))
            sps_p = ctx.enter_context(
                tc.tile_pool(name="sps", bufs=2, space="PSUM"))
            ov_p = ctx.enter_context(
                tc.tile_pool(name="ovp", bufs=1, space="PSUM"))
            sm_p = ctx.enter_context(
                tc.tile_pool(name="smp", bufs=2, space="PSUM"))

            # constant [128,128] lower-triangle mask: tri[k, q] = (k <= q)
            tri = cpool.tile([128, 128], f32)
            nc.gpsimd.memset(tri[:], 1.0)
            nc.gpsimd.affine_select(
                out=tri[:], in_=tri[:], compare_op=mybir.AluOpType.is_ge,
                fill=0.0, base=0, channel_multiplier=-1, pattern=[[1, 128]])
            tri_b = cpool.tile([128, 128], bf16)
            nc.vector.tensor_copy(tri_b[:], tri[:])

            wq_sb = cpool.tile([128, NCC, WQJ], f32r)
            nc.gpsimd.dma_start(wq_sb[:],
                                wq.rearrange("(cc p) j -> p cc j", p=128))
            wv_sb = cpool.tile([128, NCC, WVJ], f32r)
            nc.gpsimd.dma_start(wv_sb[:],
                                wv.rearrange("(cc p) j -> p cc j", p=128))
            bq_sb = cpool.tile([128, 4], f32)
            nc.sync.dma_start(bq_sb[:], bq[:, :])
            wpab_sb = cpool.tile([128, C], f32r)
            nc.gpsimd.dma_start(wpab_sb[:], wpab[:, :])
            wpc_sb = cpool.tile([64, C], f32r)
            nc.gpsimd.dma_start(wpc_sb[:], wpc[:, :])

            QT_AB = keep.tile([128, T], bf16, tag="qt_ab")
            KT_AB = keep.tile([128, T], bf16, tag="kt_ab")
            QT_C = keep.tile([128, T], bf16, tag="qt_c")
            KT_C = keep.tile([128, T], bf16, tag="kt_c")
            V3 = keep.tile([128, NKT, HPC, VP_W], bf16, tag="v3")
            # persistent ones column at [..., 64]; V copies only write 0:64
            nc.gpsimd.memset(V3[:, :, :, 64:65], 1.0)

            QK_DEST = [QT_AB, KT_AB, QT_C, KT_C]

            _xt = {}

            def qkv_dmas(tb):
                t0 = tb * QB
                xt = xtp_p.tile([128, NCC, QB], f32r, tag="xt",
                                name=f"xt{tb}")
                for cc in range(NCC):
                    nc.gpsimd.dma_start(
                        xt[:, cc, :],
                        xt_d[cc * 128:(cc + 1) * 128, t0:t0 + QB])
                _xt[tb] = xt

            def qkv_group(tb, g):
                """g in 0..3: Q/K weight block g; g in 4..7: V token tile."""
                t0 = tb * QB
                xt = _xt[tb]
                if g < 4:
                    qp = sm_p.tile([128, QB], f32, tag="small",
                                   name=f"qp{tb}_{g}")
                    for cc in range(NCC):
                        nc.tensor.matmul(
                            qp[:], wq_sb[:, cc, g * 128:(g + 1) * 128],
                            xt[:, cc, :],
                            start=(cc == 0), stop=(cc == NCC - 1))
                    nc.vector.tensor_scalar_add(
                        QK_DEST[g][:, t0:t0 + QB], qp[:], bq_sb[:, g:g + 1])
                else:
                    i = g - 4
                    vp = sm_p.tile([128, WVJ], f32, tag="small",
                                   name=f"vp{tb}_{i}")
                    for cc in range(NCC):
                        nc.tensor.matmul(
                            vp[:], xt[:, cc, i * 128:(i + 1) * 128],
                            wv_sb[:, cc, :],
                            start=(cc == 0), stop=(cc == NCC - 1))
                    dst = V3[:, tb * 4 + i, :, 0:64]
                    src = vp[:, 0:192].rearrange("p (h d) -> p h d", h=HPC)
                    nc.vector.tensor_copy(dst, src)

            def normalize(ov, dest, nm):
                # custom-DVE ops mis-read PSUM on HW: stage den via SBUF
                dsb = rr_p.tile([1, QB], f32, tag="dsb", name=f"ds{nm}")
                nc.vector.tensor_copy(dsb[:], ov[64:65, :])
                rr = rr_p.tile([1, QB], f32, tag="rr", name=f"rr{nm}")
                nc.vector.reciprocal_approx_fast(rr[:], dsb[:])
                rrb = rr_p.tile([64, QB], f32, tag="rrb", name=f"rrb{nm}")
                nc.gpsimd.partition_broadcast(rrb[:], rr[:])
                nc.vector.tensor_tensor(out=dest, in0=ov[0:64, :],
                                        in1=rrb[:], op=mult)

            def proj_unit(qb, co, onAB, onC):
                q0 = qb * QB
                yp = sm_p.tile([128, QB], f32, tag="small",
                               name=f"yp{qb}_{co}")
                nc.tensor.matmul(
                    yp[:], wpab_sb[:, co * 128:(co + 1) * 128],
                    onAB[:], start=True, stop=False)
                nc.tensor.matmul(
                    yp[:], wpc_sb[:, co * 128:(co + 1) * 128],
                    onC[:], start=False, stop=True)
                ys = ys_p.tile([128, QB], f32, tag="ys",
                               name=f"ys{qb}_{co}")
                nc.vector.tensor_copy(ys[:], yp[:])
                nc.sync.dma_start(
                    y[co * 128:(co + 1) * 128, q0:q0 + QB], ys[:])

            qkv_dmas(0)
            for g in range(8):
                qkv_group(0, g)
            prev_on = None  # (qb, onAB, onC) awaiting projection

            for tb in range(NTB):
                qb = tb
                q0 = qb * QB
                nkt = 4 * qb + 4

                # filler units spread across this block's attention
                # iterations: prev block's projection (ready immediately),
                # tb=0's own remaining QKV groups, next block's QKV groups
                units = []
                if prev_on is not None:
                    pq, pAB, pC = prev_on
                    for co in range(NCC):
                        units.append(("proj", pq, co, pAB, pC))
                    prev_on = None
                if tb + 1 < NTB:
                    qkv_dmas(tb + 1)
                    for g in range(8):
                        units.append(("qkv", tb + 1, g))
                filler = {}
                total = nkt + nkt // 2
                nun = len(units)
                for u, unit in enumerate(units):
                    pos = min(total - 1, (u * total) // nun)
                    filler.setdefault(pos, []).append(unit)

                def run_filler(pos):
                    for unit in filler.get(pos, ()):
                        if unit[0] == "qkv":
                            qkv_group(unit[1], unit[2])
                        else:
                            proj_unit(unit[1], unit[2], unit[3], unit[4])

                # heads A, B: row-group-alternated S, shared exp
                ovA = ov_p.tile([65, QB], f32, tag="ovA", name=f"ovA{qb}")
                ovB = ov_p.tile([65, QB], f32, tag="ovB", name=f"ovB{qb}")
                for kt in range(nkt):
                    k0 = kt * 128
                    m = k0 - q0
                    qo = max(0, m)  # q-trim on diagonal tiles
                    sps = sps_p.tile([128, 1024], f32, tag="sps",
                                     name=f"sAB{qb}_{kt}")
                    nc.tensor.matmul(
                        sps[:, qo:QB], KT_AB[0:64, k0:k0 + 128],
                        QT_AB[0:64, q0 + qo:q0 + QB], start=True, stop=True)
                    nc.tensor.matmul(
                        sps[:, QB + qo:2 * QB], KT_AB[64:128, k0:k0 + 128],
                        QT_AB[64:128, q0 + qo:q0 + QB],
                        start=True, stop=True)
                    pt = pt_p.tile([128, 2, QB], bf16, tag="pt")
                    spsv = sps[:].rearrange("p (h q) -> p h q", h=2)
                    nc.scalar.activation(pt[:, :, qo:QB], spsv[:, :, qo:QB],
                                         Exp, scale=0.125)
                    if 0 <= m < QB:
                        for h in range(2):
                            nc.vector.tensor_tensor(
                                out=pt[:, h, m:m + 128],
                                in0=pt[:, h, m:m + 128],
                                in1=tri_b[:], op=mult)
                    nc.tensor.matmul(ovA[:, qo:QB], V3[:, kt, 0, :],
                                     pt[:, 0, qo:QB],
                                     start=(kt == 0), stop=(kt == nkt - 1))
                    nc.tensor.matmul(ovB[:, qo:QB], V3[:, kt, 1, :],
                                     pt[:, 1, qo:QB],
                                     start=(kt == 0), stop=(kt == nkt - 1))
                    run_filler(kt)
                onAB = on_p.tile([128, QB], f32r, tag="onAB",
                                 name=f"onAB{qb}")
                normalize(ovA, onAB[0:64, :], f"A{qb}")
                normalize(ovB, onAB[64:128, :], f"B{qb}")

                # head C: row alternation via duplicated Q/K rows
                # (even kt on rows 0-63, odd kt on rows 64-127)
                ovC = ov_p.tile([65, QB], f32, tag="ovA", name=f"ovC{qb}")
                for s in range(nkt // 2):
                    kt0, kt1 = 2 * s, 2 * s + 1
                    m0 = kt0 * 128 - q0
                    m1 = m0 + 128
                    qo0, qo1 = max(0, m0), max(0, m1)
                    sps = sps_p.tile([128, 1024], f32, tag="sps",
                                     name=f"sC{qb}_{s}")
                    nc.tensor.matmul(
                        sps[:, qo0:QB],
                        KT_C[0:64, kt0 * 128:kt0 * 128 + 128],
                        QT_C[0:64, q0 + qo0:q0 + QB], start=True, stop=True)
                    # half1 writes the full shared range [qo0, QB) so the
                    # shared exp reads only written PSUM; AV reads [qo1, QB)
                    nc.tensor.matmul(
                        sps[:, QB + qo0:2 * QB],
                        KT_C[64:128, kt1 * 128:kt1 * 128 + 128],
                        QT_C[64:128, q0 + qo0:q0 + QB],
                        start=True, stop=True)
                    pt = pt_p.tile([128, 2, QB], bf16, tag="pt")
                    spsv = sps[:].rearrange("p (h q) -> p h q", h=2)
                    # shared exp at the wider range; half1's extra cols are
                    # stale-but-finite and never read by its trimmed AV
                    nc.scalar.activation(pt[:, :, qo0:QB],
                                         spsv[:, :, qo0:QB],
                                         Exp, scale=0.125)
                    for half, m in ((0, m0), (1, m1)):
                        if 0 <= m < QB:
                            nc.vector.tensor_tensor(
                                out=pt[:, half, m:m + 128],
                                in0=pt[:, half, m:m + 128],
                                in1=tri_b[:], op=mult)
                    nc.tensor.matmul(ovC[:, qo0:QB], V3[:, kt0, 2, :],
                                     pt[:, 0, qo0:QB],
                                     start=(s == 0), stop=False)
                    nc.tensor.matmul(ovC[:, qo1:QB], V3[:, kt1, 2, :],
                                     pt[:, 1, qo1:QB],
                                     start=False, stop=(s == nkt // 2 - 1))
                    run_filler(nkt + s)
                onC = on_p.tile([64, QB], f32r, tag="onC", name=f"onC{qb}")
                normalize(ovC, onC[:], f"C{qb}")
                # projection deferred: runs as filler inside the next
                # block's attention loops (keeps ACT fed at the boundary)
                prev_on = (qb, onAB, onC)

            pq, pAB, pC = prev_on
            for co in range(NCC):
                proj_unit(pq, co, pAB, pC)

    nc.finalize()
    return nc


def _core_inputs(x, w_attn, b_attn, w_proj):
    """Build the 8 per-core input maps (numpy float32)."""
    maps = []
    for core in range(NCORES):
        b = core // 4
        heads = [HPC * (core % 4) + k for k in range(HPC)]
        hA, hB, hC = heads
        qc = lambda h: slice(h * HS, (h + 1) * HS)
        kc = lambda h: slice(C + h * HS, C + (h + 1) * HS)
        vc = lambda h: slice(2 * C + h * HS, 2 * C + (h + 1) * HS)
        wqm = np.concatenate([
            w_attn[:, qc(hA)], w_attn[:, qc(hB)],
            w_attn[:, kc(hA)], w_attn[:, kc(hB)],
            w_attn[:, qc(hC)], w_attn[:, qc(hC)],
            w_attn[:, kc(hC)], w_attn[:, kc(hC)],
        ], axis=1)
        wvm = np.concatenate(
            [w_attn[:, vc(h)] for h in heads] + [np.zeros((C, 64), np.float32)],
            axis=1)
        bqm = np.zeros((128, 4), np.float32)
        bqm[0:64, 0] = b_attn[qc(hA)]
        bqm[64:128, 0] = b_attn[qc(hB)]
        bqm[0:64, 1] = b_attn[kc(hA)]
        bqm[64:128, 1] = b_attn[kc(hB)]
        bqm[0:64, 2] = b_attn[qc(hC)]
        bqm[64:128, 2] = b_attn[qc(hC)]
        bqm[0:64, 3] = b_attn[kc(hC)]
        bqm[64:128, 3] = b_attn[kc(hC)]
        wpabm = np.concatenate([w_proj[hA * HS:(hA + 1) * HS, :],
                                w_proj[hB * HS:(hB + 1) * HS, :]], axis=0)
        wpcm = w_proj[hC * HS:(hC + 1) * HS, :]
        maps.append({
            "xt": np.ascontiguousarray(x[b].T, np.float32),
            "wq": np.ascontiguousarray(wqm, np.float32),
            "wv": np.ascontiguousarray(wvm, np.float32),
            "bq": np.ascontiguousarray(bqm, np.float32),
            "wpab": np.ascontiguousarray(wpabm, np.float32),
            "wpc": np.ascontiguousarray(wpcm, np.float32),
        })
    return maps


def run_cores(in_maps, trace=False):
    from concourse import bass_utils
    if "nc" not in _CACHE:
        _CACHE["nc"] = _build()
    return bass_utils.run_bass_kernel_spmd(
        _CACHE["nc"], in_maps, list(range(NCORES)), trace=trace)


def kernel(x, w_attn, b_attn, w_proj, b_proj):
    x = np.asarray(x, np.float32)
    w_attn = np.asarray(w_attn, np.float32)
    b_attn = np.asarray(b_attn, np.float32)
    w_proj = np.asarray(w_proj, np.float32)
    b_proj = np.asarray(b_proj, np.float32)

    # V-bias folds exactly into a constant row: sum_k P/den = 1, so
    # O_h = AV_h/den + bv_h and its projection adds bv_h @ W_h.
    b_eff = b_proj.astype(np.float64).copy()
    for h in range(NH):
        bv = b_attn[2 * C + h * HS:2 * C + (h + 1) * HS].astype(np.float64)
        b_eff += bv @ w_proj[h * HS:(h + 1) * HS, :].astype(np.float64)

    in_maps = _core_inputs(x, w_attn, b_attn, w_proj)
    res = run_cores(in_maps)
    y = np.zeros((B, T, C), np.float32)
    for b in range(B):
        acc = np.zeros((C, T), np.float64)
        for core in range(4 * b, 4 * b + 4):
            acc += res.results[core]["y"].astype(np.float64)
        y[b] = (acc.T + b_eff[None, :]).astype(np.float32)
    return y


# revision 11
# speedup vs baseline: 1.6088x; 1.0631x over previous
"""Causal self-attention kernel for 8 TRN2 NeuronCores (v3).

Problem (hardcoded): B=2, T=4096, C=768, NH=12, HS=64.
  qkv = x @ w_attn + b_attn; per-head causal softmax attention;
  y = att_out @ w_proj + b_proj

Sharding: 24 (batch, head) units over 8 cores -> 3 heads per core.
  cores 0..3: batch 0, heads (0,1,2), (3,4,5), (6,7,8), (9,10,11)
  cores 4..7: batch 1, same head split.
Each core computes a partial y^T [C, T]; the host sums partials per batch
and adds b_eff = b_proj + sum_h bv_h @ w_proj[h] (the V-bias contribution
commutes with the softmax average exactly: sum_k P/den = 1).

v3 structure (ScalarE exp is the hard floor at ~1 elem/cycle/lane;
everything else is arranged to keep ACT saturated and PE warm):
  - The QKV matmul groups for block tb+1 are emitted INTERLEAVED into the
    attention kt-loops of block tb, so the PE always has independent work
    during exp waits and ScalarE never drains at phase boundaries (v2
    showed ~22us ACT idle per tb and 14 HAM re-throttles without this).
  - Diagonal k-tiles are q-trimmed: S/exp/AV only touch q >= k_tile_base,
    the causal mask multiply shrinks to a constant [128,128] triangle.
  - V is computed directly in [token, dim] layout (stationary = x^T chunk,
    moving = wv) - no PE transposes (transpose-mode doesn't count as
    PE-busy for the HAM clock gate).
  - Normalize: den row staged to SBUF (custom-DVE ops mis-read PSUM on HW),
    reciprocal_approx_fast, gpsimd partition_broadcast, one DVE multiply.
  - Projection packs heads A,B into one 128-contraction matmul; head C's
    Q/K are duplicated ([qC|qC], [kC|kC]) so S_C pairs PE row groups.
"""

import numpy as np

B, T, C, NH = 2, 4096, 768, 12
HS = C // NH          # 64
NCORES = 8
HPC = 3               # heads per core
QB = 512              # q block (moving dim)
NQB = T // QB         # 8
NKT = T // 128        # 32 k-tiles
NTB = T // QB         # t-blocks
NCC = C // 128        # 6 contraction chunks
VP_W = 65             # per-head V block: 64 dims + ones column
WQJ = 4 * 128         # q/k weight blocks: [qA|qB | kA|kB | qC|qC | kC|kC]
WVJ = 192             # v moving width (3 heads x 64)

_CACHE = {}


def _build():
    import contextlib
    import concourse.bacc as bacc
    import concourse.mybir as mybir
    from concourse.tile import TileContext

    f32 = mybir.dt.float32
    f32r = mybir.dt.float32r
    bf16 = mybir.dt.bfloat16
    Exp = mybir.ActivationFunctionType.Exp
    mult = mybir.AluOpType.mult

    nc = bacc.Bacc(trn_type="TRN2")

    xt_d = nc.dram_tensor("xt", [C, T], bf16, kind="ExternalInput")
    wq = nc.dram_tensor("wq", [C, WQJ], bf16, kind="ExternalInput")
    wv = nc.dram_tensor("wv", [C, WVJ], bf16, kind="ExternalInput")
    bq = nc.dram_tensor("bq", [128, 4], f32, kind="ExternalInput")
    wpab = nc.dram_tensor("wpab", [128, C], bf16, kind="ExternalInput")
    wpc = nc.dram_tensor("wpc", [64, C], bf16, kind="ExternalInput")
    y = nc.dram_tensor("y", [C, T], f32, kind="ExternalOutput")

    with TileContext(nc) as tc, nc.allow_low_precision("f32r kernel"):
        with contextlib.ExitStack() as ctx:
            cpool = ctx.enter_context(tc.tile_pool(name="const", bufs=1))
            keep = ctx.enter_context(tc.tile_pool(name="keep", bufs=1))
            xtp_p = ctx.enter_context(tc.tile_pool(name="xtp", bufs=2))
            pt_p = ctx.enter_context(tc.tile_pool(name="ptp", bufs=4))
            on_p = ctx.enter_context(tc.tile_pool(name="onp", bufs=2))
            rr_p = ctx.enter_context(tc.tile_pool(name="rrp", bufs=2))
            ys_p = ctx.enter_context(tc.tile_pool(name="ysp", bufs=2))
            sps_p = ctx.enter_context(
                tc.tile_pool(name="sps", bufs=2, space="PSUM"))
            ov_p = ctx.enter_context(
                tc.tile_pool(name="ovp", bufs=1, space="PSUM"))
            sm_p = ctx.enter_context(
                tc.tile_pool(name="smp", bufs=2, space="PSUM"))

            # constant [128,128] lower-triangle mask: tri[k, q] = (k <= q)
            tri = cpool.tile([128, 128], f32)
            nc.gpsimd.memset(tri[:], 1.0)
            nc.gpsimd.affine_select(
                out=tri[:], in_=tri[:], compare_op=mybir.AluOpType.is_ge,
                fill=0.0, base=0, channel_multiplier=-1, pattern=[[1, 128]])
            tri_b = cpool.tile([128, 128], bf16)
            nc.vector.tensor_copy(tri_b[:], tri[:])

            wq_sb = cpool.tile([128, NCC, WQJ], bf16)
            for cc in range(NCC):
                nc.gpsimd.dma_start(wq_sb[:, cc, :],
                                    wq[cc * 128:(cc + 1) * 128, :])
            wv_sb = cpool.tile([128, NCC, WVJ], bf16)
            nc.gpsimd.dma_start(wv_sb[:],
                                wv.rearrange("(cc p) j -> p cc j", p=128))
            bq_sb = cpool.tile([128, 4], f32)
            nc.sync.dma_start(bq_sb[:], bq[:, :])
            wpab_sb = cpool.tile([128, C], bf16)
            nc.gpsimd.dma_start(wpab_sb[:], wpab[:, :])
            wpc_sb = cpool.tile([64, C], bf16)
            nc.gpsimd.dma_start(wpc_sb[:], wpc[:, :])

            QT_AB = keep.tile([128, T], bf16, tag="qt_ab")
            KT_AB = keep.tile([128, T], bf16, tag="kt_ab")
            QT_C = keep.tile([128, T], bf16, tag="qt_c")
            KT_C = keep.tile([128, T], bf16, tag="kt_c")
            V3 = keep.tile([128, NKT, HPC, VP_W], bf16, tag="v3")
            # persistent ones column at [..., 64]; V copies only write 0:64
            nc.gpsimd.memset(V3[:, :, :, 64:65], 1.0)

            QK_DEST = [QT_AB, KT_AB, QT_C, KT_C]

            _xt = {}

            def qkv_dmas(tb):
                t0 = tb * QB
                xt = xtp_p.tile([128, NCC, QB], bf16, tag="xt",
                                name=f"xt{tb}")
                for cc in range(NCC):
                    nc.gpsimd.dma_start(
                        xt[:, cc, :],
                        xt_d[cc * 128:(cc + 1) * 128, t0:t0 + QB])
                _xt[tb] = xt

            def qkv_group(tb, g):
                """g in 0..3: Q/K weight block g; g in 4..7: V token tile."""
                t0 = tb * QB
                xt = _xt[tb]
                if g < 4:
                    qp = sm_p.tile([128, QB], f32, tag="small",
                                   name=f"qp{tb}_{g}")
                    for cc in range(NCC):
                        nc.tensor.matmul(
                            qp[:], wq_sb[:, cc, g * 128:(g + 1) * 128],
                            xt[:, cc, :],
                            start=(cc == 0), stop=(cc == NCC - 1))
                    nc.vector.tensor_scalar_add(
                        QK_DEST[g][:, t0:t0 + QB], qp[:], bq_sb[:, g:g + 1])
                else:
                    i = g - 4
                    vp = sm_p.tile([128, WVJ], f32, tag="small",
                                   name=f"vp{tb}_{i}")
                    for cc in range(NCC):
                        nc.tensor.matmul(
                            vp[:], xt[:, cc, i * 128:(i + 1) * 128],
                            wv_sb[:, cc, :],
                            start=(cc == 0), stop=(cc == NCC - 1))
                    dst = V3[:, tb * 4 + i, :, 0:64]
                    src = vp[:, :].rearrange("p (h d) -> p h d", h=HPC)
                    nc.vector.tensor_copy(dst, src)

            def normalize(ov, dest, nm):
                # one copy stages ov to SBUF and frees its PSUM bank; the
                # custom-DVE recip needs an SBUF source on HW anyway
                osb = rr_p.tile([65, QB], f32, tag="osb", name=f"os{nm}")
                nc.vector.tensor_copy(osb[:], ov[:])
                # the custom-DVE recip needs a partition-0 SBUF source on HW
                dsb = rr_p.tile([1, QB], f32, tag="dsb", name=f"ds{nm}")
                nc.vector.tensor_copy(dsb[:], ov[64:65, :])
                rr = rr_p.tile([1, QB], f32, tag="rr", name=f"rr{nm}")
                nc.vector.reciprocal_approx_fast(rr[:], dsb[:])
                rrb = rr_p.tile([64, QB], f32, tag="rrb", name=f"rrb{nm}")
                nc.gpsimd.partition_broadcast(rrb[:], rr[:])
                nc.vector.tensor_tensor(out=dest, in0=osb[0:64, :],
                                        in1=rrb[:], op=mult)

            def proj_unit(qb, co, onAB, onC):
                q0 = qb * QB
                yp = sm_p.tile([128, QB], f32, tag="small",
                               name=f"yp{qb}_{co}")
                nc.tensor.matmul(
                    yp[:], wpab_sb[:, co * 128:(co + 1) * 128],
                    onAB[:], start=True, stop=False)
                nc.tensor.matmul(
                    yp[:], wpc_sb[:, co * 128:(co + 1) * 128],
                    onC[:], start=False, stop=True)
                ys = ys_p.tile([128, QB], f32, tag="ys",
                               name=f"ys{qb}_{co}")
                nc.vector.tensor_copy(ys[:], yp[:])
                nc.sync.dma_start(
                    y[co * 128:(co + 1) * 128, q0:q0 + QB], ys[:])

            qkv_dmas(0)
            for g in range(8):
                qkv_group(0, g)
            prev_on = None  # (qb, onAB, onC) awaiting projection

            for tb in range(NTB):
                qb = tb
                q0 = qb * QB
                nkt = 4 * qb + 4

                # filler units spread across this block's attention
                # iterations: prev block's projection (ready immediately),
                # tb=0's own remaining QKV groups, next block's QKV groups
                units = []
                if prev_on is not None:
                    pq, pAB, pC = prev_on
                    for co in range(NCC):
                        units.append(("proj", pq, co, pAB, pC))
                    prev_on = None
                if tb + 1 < NTB:
                    qkv_dmas(tb + 1)
                    for g in range(8):
                        units.append(("qkv", tb + 1, g))
                filler = {}
                total = nkt + nkt // 2
                nun = len(units)
                for u, unit in enumerate(units):
                    pos = min(total - 1, (u * total) // nun)
                    filler.setdefault(pos, []).append(unit)

                def run_filler(pos):
                    for unit in filler.get(pos, ()):
                        if unit[0] == "qkv":
                            qkv_group(unit[1], unit[2])
                        else:
                            proj_unit(unit[1], unit[2], unit[3], unit[4])

                # heads A, B: row-group-alternated S, shared exp
                ovA = ov_p.tile([65, QB], f32, tag="ovA", name=f"ovA{qb}")
                ovB = ov_p.tile([65, QB], f32, tag="ovB", name=f"ovB{qb}")
                for kt in range(nkt):
                    k0 = kt * 128
                    m = k0 - q0
                    qo = max(0, m)  # q-trim on diagonal tiles
                    sps = sps_p.tile([128, 1024], f32, tag="sps",
                                     name=f"sAB{qb}_{kt}")
                    nc.tensor.matmul(
                        sps[:, qo:QB], KT_AB[0:64, k0:k0 + 128],
                        QT_AB[0:64, q0 + qo:q0 + QB], start=True, stop=True)
                    nc.tensor.matmul(
                        sps[:, QB + qo:2 * QB], KT_AB[64:128, k0:k0 + 128],
                        QT_AB[64:128, q0 + qo:q0 + QB],
                        start=True, stop=True)
                    pt = pt_p.tile([128, 2, QB], bf16, tag="pt")
                    spsv = sps[:].rearrange("p (h q) -> p h q", h=2)
                    nc.scalar.activation(pt[:, :, qo:QB], spsv[:, :, qo:QB],
                                         Exp, scale=0.125)
                    if 0 <= m < QB:
                        for h in range(2):
                            nc.vector.tensor_tensor(
                                out=pt[:, h, m:m + 128],
                                in0=pt[:, h, m:m + 128],
                                in1=tri_b[:], op=mult)
                    nc.tensor.matmul(ovA[:, qo:QB], V3[:, kt, 0, :],
                                     pt[:, 0, qo:QB],
                                     start=(kt == 0), stop=(kt == nkt - 1))
                    nc.tensor.matmul(ovB[:, qo:QB], V3[:, kt, 1, :],
                                     pt[:, 1, qo:QB],
                                     start=(kt == 0), stop=(kt == nkt - 1))
                    run_filler(kt)
                onAB = on_p.tile([128, QB], bf16, tag="onAB",
                                 name=f"onAB{qb}")
                normalize(ovA, onAB[0:64, :], f"A{qb}")
                normalize(ovB, onAB[64:128, :], f"B{qb}")

                # head C: row alternation via duplicated Q/K rows
                # (even kt on rows 0-63, odd kt on rows 64-127)
                ovC = ov_p.tile([65, QB], f32, tag="ovA", name=f"ovC{qb}")
                for s in range(nkt // 2):
                    kt0, kt1 = 2 * s, 2 * s + 1
                    m0 = kt0 * 128 - q0
                    m1 = m0 + 128
                    qo0, qo1 = max(0, m0), max(0, m1)
                    sps = sps_p.tile([128, 1024], f32, tag="sps",
                                     name=f"sC{qb}_{s}")
                    nc.tensor.matmul(
                        sps[:, qo0:QB],
                        KT_C[0:64, kt0 * 128:kt0 * 128 + 128],
                        QT_C[0:64, q0 + qo0:q0 + QB], start=True, stop=True)
                    # half1 writes the full shared range [qo0, QB) so the
                    # shared exp reads only written PSUM; AV reads [qo1, QB)
                    nc.tensor.matmul(
                        sps[:, QB + qo0:2 * QB],
                        KT_C[64:128, kt1 * 128:kt1 * 128 + 128],
                        QT_C[64:128, q0 + qo0:q0 + QB],
                        start=True, stop=True)
                    pt = pt_p.tile([128, 2, QB], bf16, tag="pt")
                    spsv = sps[:].rearrange("p (h q) -> p h q", h=2)
                    # shared exp at the wider range; half1's extra cols are
                    # stale-but-finite and never read by its trimmed AV
                    nc.scalar.activation(pt[:, :, qo0:QB],
                                         spsv[:, :, qo0:QB],
                                         Exp, scale=0.125)
                    for half, m in ((0, m0), (1, m1)):
                        if 0 <= m < QB:
                            nc.vector.tensor_tensor(
                                out=pt[:, half, m:m + 128],
                                in0=pt[:, half, m:m + 128],
                                in1=tri_b[:], op=mult)
                    nc.tensor.matmul(ovC[:, qo0:QB], V3[:, kt0, 2, :],
                                     pt[:, 0, qo0:QB],
                                     start=(s == 0), stop=False)
                    nc.tensor.matmul(ovC[:, qo1:QB], V3[:, kt1, 2, :],
                                     pt[:, 1, qo1:QB],
                                     start=False, stop=(s == nkt // 2 - 1))
                    run_filler(nkt + s)
                onC = on_p.tile([64, QB], bf16, tag="onC", name=f"onC{qb}")
                normalize(ovC, onC[:], f"C{qb}")
                # projection deferred: runs as filler inside the next
                # block's attention loops (keeps ACT fed at the boundary)
                prev_on = (qb, onAB, onC)

            pq, pAB, pC = prev_on
            for co in range(NCC):
                proj_unit(pq, co, pAB, pC)

    nc.finalize()
    return nc


def _core_inputs(x, w_attn, b_attn, w_proj):
    """Build the 8 per-core input maps (numpy float32)."""
    maps = []
    for core in range(NCORES):
        b = core // 4
        heads = [HPC * (core % 4) + k for k in range(HPC)]
        hA, hB, hC = heads
        qc = lambda h: slice(h * HS, (h + 1) * HS)
        kc = lambda h: slice(C + h * HS, C + (h + 1) * HS)
        vc = lambda h: slice(2 * C + h * HS, 2 * C + (h + 1) * HS)
        wqm = np.concatenate([
            w_attn[:, qc(hA)], w_attn[:, qc(hB)],
            w_attn[:, kc(hA)], w_attn[:, kc(hB)],
            w_attn[:, qc(hC)], w_attn[:, qc(hC)],
            w_attn[:, kc(hC)], w_attn[:, kc(hC)],
        ], axis=1)
        wvm = np.concatenate([w_attn[:, vc(h)] for h in heads], axis=1)
        bqm = np.zeros((128, 4), np.float32)
        bqm[0:64, 0] = b_attn[qc(hA)]
        bqm[64:128, 0] = b_attn[qc(hB)]
        bqm[0:64, 1] = b_attn[kc(hA)]
        bqm[64:128, 1] = b_attn[kc(hB)]
        bqm[0:64, 2] = b_attn[qc(hC)]
        bqm[64:128, 2] = b_attn[qc(hC)]
        bqm[0:64, 3] = b_attn[kc(hC)]
        bqm[64:128, 3] = b_attn[kc(hC)]
        wpabm = np.concatenate([w_proj[hA * HS:(hA + 1) * HS, :],
                                w_proj[hB * HS:(hB + 1) * HS, :]], axis=0)
        wpcm = w_proj[hC * HS:(hC + 1) * HS, :]
        import ml_dtypes
        bf = ml_dtypes.bfloat16
        maps.append({
            "xt": np.ascontiguousarray(x[b].T).astype(bf),
            "wq": np.ascontiguousarray(wqm).astype(bf),
            "wv": np.ascontiguousarray(wvm).astype(bf),
            "bq": np.ascontiguousarray(bqm, np.float32),
            "wpab": np.ascontiguousarray(wpabm).astype(bf),
            "wpc": np.ascontiguousarray(wpcm).astype(bf),
        })
    return maps


def run_cores(in_maps, trace=False):
    from concourse import bass_utils
    if "nc" not in _CACHE:
        _CACHE["nc"] = _build()
    return bass_utils.run_bass_kernel_spmd(
        _CACHE["nc"], in_maps, list(range(NCORES)), trace=trace)


def kernel(x, w_attn, b_attn, w_proj, b_proj):
    x = np.asarray(x, np.float32)
    w_attn = np.asarray(w_attn, np.float32)
    b_attn = np.asarray(b_attn, np.float32)
    w_proj = np.asarray(w_proj, np.float32)
    b_proj = np.asarray(b_proj, np.float32)

    # V-bias folds exactly into a constant row: sum_k P/den = 1, so
    # O_h = AV_h/den + bv_h and its projection adds bv_h @ W_h.
    b_eff = b_proj.astype(np.float64).copy()
    for h in range(NH):
        bv = b_attn[2 * C + h * HS:2 * C + (h + 1) * HS].astype(np.float64)
        b_eff += bv @ w_proj[h * HS:(h + 1) * HS, :].astype(np.float64)

    in_maps = _core_inputs(x, w_attn, b_attn, w_proj)
    res = run_cores(in_maps)
    y = np.zeros((B, T, C), np.float32)
    for b in range(B):
        acc = np.zeros((C, T), np.float64)
        for core in range(4 * b, 4 * b + 4):
            acc += res.results[core]["y"].astype(np.float64)
        y[b] = (acc.T + b_eff[None, :]).astype(np.float32)
    return y


# revision 12
# speedup vs baseline: 1.6107x; 1.0011x over previous
"""Causal self-attention kernel for 8 TRN2 NeuronCores (v3).

Problem (hardcoded): B=2, T=4096, C=768, NH=12, HS=64.
  qkv = x @ w_attn + b_attn; per-head causal softmax attention;
  y = att_out @ w_proj + b_proj

Sharding: 24 (batch, head) units over 8 cores -> 3 heads per core.
  cores 0..3: batch 0, heads (0,1,2), (3,4,5), (6,7,8), (9,10,11)
  cores 4..7: batch 1, same head split.
Each core computes a partial y^T [C, T]; the host sums partials per batch
and adds b_eff = b_proj + sum_h bv_h @ w_proj[h] (the V-bias contribution
commutes with the softmax average exactly: sum_k P/den = 1).

v3 structure (ScalarE exp is the hard floor at ~1 elem/cycle/lane;
everything else is arranged to keep ACT saturated and PE warm):
  - The QKV matmul groups for block tb+1 are emitted INTERLEAVED into the
    attention kt-loops of block tb, so the PE always has independent work
    during exp waits and ScalarE never drains at phase boundaries (v2
    showed ~22us ACT idle per tb and 14 HAM re-throttles without this).
  - Diagonal k-tiles are q-trimmed: S/exp/AV only touch q >= k_tile_base,
    the causal mask multiply shrinks to a constant [128,128] triangle.
  - V is computed directly in [token, dim] layout (stationary = x^T chunk,
    moving = wv) - no PE transposes (transpose-mode doesn't count as
    PE-busy for the HAM clock gate).
  - Normalize: den row staged to SBUF (custom-DVE ops mis-read PSUM on HW),
    reciprocal_approx_fast, gpsimd partition_broadcast, one DVE multiply.
  - Projection packs heads A,B into one 128-contraction matmul; head C's
    Q/K are duplicated ([qC|qC], [kC|kC]) so S_C pairs PE row groups.
"""

import numpy as np

B, T, C, NH = 2, 4096, 768, 12
HS = C // NH          # 64
NCORES = 8
HPC = 3               # heads per core
QB = 512              # q block (moving dim)
NQB = T // QB         # 8
NKT = T // 128        # 32 k-tiles
NTB = T // QB         # t-blocks
NCC = C // 128        # 6 contraction chunks
VP_W = 65             # per-head V block: 64 dims + ones column
WQJ = 4 * 128         # q/k weight blocks: [qA|qB | kA|kB | qC|qC | kC|kC]
WVJ = 192             # v moving width (3 heads x 64)

_CACHE = {}


def _build():
    import contextlib
    import concourse.bacc as bacc
    import concourse.mybir as mybir
    from concourse.tile import TileContext

    f32 = mybir.dt.float32
    f32r = mybir.dt.float32r
    bf16 = mybir.dt.bfloat16
    Exp = mybir.ActivationFunctionType.Exp
    mult = mybir.AluOpType.mult

    nc = bacc.Bacc(trn_type="TRN2")

    xt_d = nc.dram_tensor("xt", [C, T], bf16, kind="ExternalInput")
    wq = nc.dram_tensor("wq", [C, WQJ], bf16, kind="ExternalInput")
    wv = nc.dram_tensor("wv", [C, WVJ], bf16, kind="ExternalInput")
    bq = nc.dram_tensor("bq", [128, 4], f32, kind="ExternalInput")
    wpab = nc.dram_tensor("wpab", [128, C], bf16, kind="ExternalInput")
    wpc = nc.dram_tensor("wpc", [64, C], bf16, kind="ExternalInput")
    y = nc.dram_tensor("y", [C, T], f32, kind="ExternalOutput")

    with TileContext(nc) as tc, nc.allow_low_precision("f32r kernel"):
        with contextlib.ExitStack() as ctx:
            cpool = ctx.enter_context(tc.tile_pool(name="const", bufs=1))
            keep = ctx.enter_context(tc.tile_pool(name="keep", bufs=1))
            xtp_p = ctx.enter_context(tc.tile_pool(name="xtp", bufs=2))
            pt_p = ctx.enter_context(tc.tile_pool(name="ptp", bufs=4))
            on_p = ctx.enter_context(tc.tile_pool(name="onp", bufs=2))
            rr_p = ctx.enter_context(tc.tile_pool(name="rrp", bufs=2))
            ys_p = ctx.enter_context(tc.tile_pool(name="ysp", bufs=2))
            sps_p = ctx.enter_context(
                tc.tile_pool(name="sps", bufs=2, space="PSUM"))
            ov_p = ctx.enter_context(
                tc.tile_pool(name="ovp", bufs=1, space="PSUM"))
            sm_p = ctx.enter_context(
                tc.tile_pool(name="smp", bufs=2, space="PSUM"))

            # constant [128,128] lower-triangle mask: tri[k, q] = (k <= q)
            tri = cpool.tile([128, 128], f32)
            nc.gpsimd.memset(tri[:], 1.0)
            nc.gpsimd.affine_select(
                out=tri[:], in_=tri[:], compare_op=mybir.AluOpType.is_ge,
                fill=0.0, base=0, channel_multiplier=-1, pattern=[[1, 128]])
            tri_b = cpool.tile([128, 128], bf16)
            nc.vector.tensor_copy(tri_b[:], tri[:])

            wq_sb = cpool.tile([128, NCC, WQJ], bf16)
            for cc in range(NCC):
                nc.gpsimd.dma_start(wq_sb[:, cc, :],
                                    wq[cc * 128:(cc + 1) * 128, :])
            wv_sb = cpool.tile([128, NCC, WVJ], bf16)
            nc.sync.dma_start(wv_sb[:],
                              wv.rearrange("(cc p) j -> p cc j", p=128))
            bq_sb = cpool.tile([128, 4], f32)
            nc.sync.dma_start(bq_sb[:], bq[:, :])
            wpab_sb = cpool.tile([128, C], bf16)
            wpc_sb = cpool.tile([64, C], bf16)

            QT_AB = keep.tile([128, T], bf16, tag="qt_ab")
            KT_AB = keep.tile([128, T], bf16, tag="kt_ab")
            QT_C = keep.tile([128, T], bf16, tag="qt_c")
            KT_C = keep.tile([128, T], bf16, tag="kt_c")
            V3 = keep.tile([128, NKT, HPC, VP_W], bf16, tag="v3")
            # persistent ones column at [..., 64]; V copies only write 0:64
            nc.gpsimd.memset(V3[:, :, :, 64:65], 1.0)

            QK_DEST = [QT_AB, KT_AB, QT_C, KT_C]

            _xt = {}

            def qkv_dmas(tb):
                t0 = tb * QB
                xt = xtp_p.tile([128, NCC, QB], bf16, tag="xt",
                                name=f"xt{tb}")
                eng = nc.sync if tb == 0 else nc.gpsimd
                for cc in range(NCC):
                    eng.dma_start(
                        xt[:, cc, :],
                        xt_d[cc * 128:(cc + 1) * 128, t0:t0 + QB])
                _xt[tb] = xt

            def qkv_group(tb, g):
                """g in 0..3: Q/K weight block g; g in 4..7: V token tile."""
                t0 = tb * QB
                xt = _xt[tb]
                if g < 4:
                    qp = sm_p.tile([128, QB], f32, tag="small",
                                   name=f"qp{tb}_{g}")
                    for cc in range(NCC):
                        nc.tensor.matmul(
                            qp[:], wq_sb[:, cc, g * 128:(g + 1) * 128],
                            xt[:, cc, :],
                            start=(cc == 0), stop=(cc == NCC - 1))
                    nc.vector.tensor_scalar_add(
                        QK_DEST[g][:, t0:t0 + QB], qp[:], bq_sb[:, g:g + 1])
                else:
                    i = g - 4
                    vp = sm_p.tile([128, WVJ], f32, tag="small",
                                   name=f"vp{tb}_{i}")
                    for cc in range(NCC):
                        nc.tensor.matmul(
                            vp[:], xt[:, cc, i * 128:(i + 1) * 128],
                            wv_sb[:, cc, :],
                            start=(cc == 0), stop=(cc == NCC - 1))
                    dst = V3[:, tb * 4 + i, :, 0:64]
                    src = vp[:, :].rearrange("p (h d) -> p h d", h=HPC)
                    nc.vector.tensor_copy(dst, src)

            def normalize(ov, dest, nm):
                # one copy stages ov to SBUF and frees its PSUM bank; the
                # custom-DVE recip needs an SBUF source on HW anyway
                osb = rr_p.tile([65, QB], f32, tag="osb", name=f"os{nm}")
                nc.vector.tensor_copy(osb[:], ov[:])
                # the custom-DVE recip needs a partition-0 SBUF source on HW
                dsb = rr_p.tile([1, QB], f32, tag="dsb", name=f"ds{nm}")
                nc.vector.tensor_copy(dsb[:], ov[64:65, :])
                rr = rr_p.tile([1, QB], f32, tag="rr", name=f"rr{nm}")
                nc.vector.reciprocal_approx_fast(rr[:], dsb[:])
                rrb = rr_p.tile([64, QB], f32, tag="rrb", name=f"rrb{nm}")
                nc.gpsimd.partition_broadcast(rrb[:], rr[:])
                nc.vector.tensor_tensor(out=dest, in0=osb[0:64, :],
                                        in1=rrb[:], op=mult)

            def proj_unit(qb, co, onAB, onC):
                q0 = qb * QB
                yp = sm_p.tile([128, QB], f32, tag="small",
                               name=f"yp{qb}_{co}")
                nc.tensor.matmul(
                    yp[:], wpab_sb[:, co * 128:(co + 1) * 128],
                    onAB[:], start=True, stop=False)
                nc.tensor.matmul(
                    yp[:], wpc_sb[:, co * 128:(co + 1) * 128],
                    onC[:], start=False, stop=True)
                ys = ys_p.tile([128, QB], f32, tag="ys",
                               name=f"ys{qb}_{co}")
                nc.vector.tensor_copy(ys[:], yp[:])
                nc.sync.dma_start(
                    y[co * 128:(co + 1) * 128, q0:q0 + QB], ys[:])

            qkv_dmas(0)
            # remaining proj weights can load behind the first x/wq chunks
            nc.gpsimd.dma_start(wpab_sb[:], wpab[:, :])
            nc.gpsimd.dma_start(wpc_sb[:], wpc[:, :])
            qkv_group(0, 0)
            qkv_group(0, 1)
            prev_on = None  # (qb, onAB, onC) awaiting projection

            for tb in range(NTB):
                qb = tb
                q0 = qb * QB
                nkt = 4 * qb + 4

                # filler units spread across this block's attention
                # iterations: prev block's projection (ready immediately),
                # tb=0's own remaining QKV groups, next block's QKV groups
                units = []
                if prev_on is not None:
                    pq, pAB, pC = prev_on
                    for co in range(NCC):
                        units.append(("proj", pq, co, pAB, pC))
                    prev_on = None
                if tb + 1 < NTB:
                    qkv_dmas(tb + 1)
                    for g in range(8):
                        units.append(("qkv", tb + 1, g))
                filler = {}
                total = nkt + nkt // 2
                nun = max(1, len(units))
                for u, unit in enumerate(units):
                    pos = min(total - 1, (u * total) // nun)
                    filler.setdefault(pos, []).append(unit)

                def run_filler(pos):
                    for unit in filler.get(pos, ()):
                        if unit[0] == "qkv":
                            qkv_group(unit[1], unit[2])
                        else:
                            proj_unit(unit[1], unit[2], unit[3], unit[4])

                def pre_av(kt):
                    # tb=0 bootstrap: this kt's V tile right before its AV,
                    # C's Q/K blocks behind the first two iterations
                    if tb == 0:
                        qkv_group(0, 4 + kt)
                        if kt < 2:
                            qkv_group(0, 2 + kt)

                # heads A, B: row-group-alternated S, shared exp
                ovA = ov_p.tile([65, QB], f32, tag="ovA", name=f"ovA{qb}")
                ovB = ov_p.tile([65, QB], f32, tag="ovB", name=f"ovB{qb}")
                for kt in range(nkt):
                    k0 = kt * 128
                    m = k0 - q0
                    qo = max(0, m)  # q-trim on diagonal tiles
                    sps = sps_p.tile([128, 1024], f32, tag="sps",
                                     name=f"sAB{qb}_{kt}")
                    nc.tensor.matmul(
                        sps[:, qo:QB], KT_AB[0:64, k0:k0 + 128],
                        QT_AB[0:64, q0 + qo:q0 + QB], start=True, stop=True)
                    nc.tensor.matmul(
                        sps[:, QB + qo:2 * QB], KT_AB[64:128, k0:k0 + 128],
                        QT_AB[64:128, q0 + qo:q0 + QB],
                        start=True, stop=True)
                    pt = pt_p.tile([128, 2, QB], bf16, tag="pt")
                    spsv = sps[:].rearrange("p (h q) -> p h q", h=2)
                    nc.scalar.activation(pt[:, :, qo:QB], spsv[:, :, qo:QB],
                                         Exp, scale=0.125)
                    if 0 <= m < QB:
                        for h in range(2):
                            nc.vector.tensor_tensor(
                                out=pt[:, h, m:m + 128],
                                in0=pt[:, h, m:m + 128],
                                in1=tri_b[:], op=mult)
                    pre_av(kt)
                    nc.tensor.matmul(ovA[:, qo:QB], V3[:, kt, 0, :],
                                     pt[:, 0, qo:QB],
                                     start=(kt == 0), stop=(kt == nkt - 1))
                    nc.tensor.matmul(ovB[:, qo:QB], V3[:, kt, 1, :],
                                     pt[:, 1, qo:QB],
                                     start=(kt == 0), stop=(kt == nkt - 1))
                    run_filler(kt)
                onAB = on_p.tile([128, QB], bf16, tag="onAB",
                                 name=f"onAB{qb}")
                normalize(ovA, onAB[0:64, :], f"A{qb}")
                normalize(ovB, onAB[64:128, :], f"B{qb}")

                # head C: row alternation via duplicated Q/K rows
                # (even kt on rows 0-63, odd kt on rows 64-127)
                ovC = ov_p.tile([65, QB], f32, tag="ovA", name=f"ovC{qb}")
                for s in range(nkt // 2):
                    kt0, kt1 = 2 * s, 2 * s + 1
                    m0 = kt0 * 128 - q0
                    m1 = m0 + 128
                    qo0, qo1 = max(0, m0), max(0, m1)
                    sps = sps_p.tile([128, 1024], f32, tag="sps",
                                     name=f"sC{qb}_{s}")
                    nc.tensor.matmul(
                        sps[:, qo0:QB],
                        KT_C[0:64, kt0 * 128:kt0 * 128 + 128],
                        QT_C[0:64, q0 + qo0:q0 + QB], start=True, stop=True)
                    # half1 writes the full shared range [qo0, QB) so the
                    # shared exp reads only written PSUM; AV reads [qo1, QB)
                    nc.tensor.matmul(
                        sps[:, QB + qo0:2 * QB],
                        KT_C[64:128, kt1 * 128:kt1 * 128 + 128],
                        QT_C[64:128, q0 + qo0:q0 + QB],
                        start=True, stop=True)
                    pt = pt_p.tile([128, 2, QB], bf16, tag="pt")
                    spsv = sps[:].rearrange("p (h q) -> p h q", h=2)
                    # shared exp at the wider range; half1's extra cols are
                    # stale-but-finite and never read by its trimmed AV
                    nc.scalar.activation(pt[:, :, qo0:QB],
                                         spsv[:, :, qo0:QB],
                                         Exp, scale=0.125)
                    for half, m in ((0, m0), (1, m1)):
                        if 0 <= m < QB:
                            nc.vector.tensor_tensor(
                                out=pt[:, half, m:m + 128],
                                in0=pt[:, half, m:m + 128],
                                in1=tri_b[:], op=mult)
                    nc.tensor.matmul(ovC[:, qo0:QB], V3[:, kt0, 2, :],
                                     pt[:, 0, qo0:QB],
                                     start=(s == 0), stop=False)
                    nc.tensor.matmul(ovC[:, qo1:QB], V3[:, kt1, 2, :],
                                     pt[:, 1, qo1:QB],
                                     start=False, stop=(s == nkt // 2 - 1))
                    run_filler(nkt + s)
                onC = on_p.tile([64, QB], bf16, tag="onC", name=f"onC{qb}")
                normalize(ovC, onC[:], f"C{qb}")
                # projection deferred: runs as filler inside the next
                # block's attention loops (keeps ACT fed at the boundary)
                prev_on = (qb, onAB, onC)

            pq, pAB, pC = prev_on
            for co in range(NCC):
                proj_unit(pq, co, pAB, pC)

    nc.finalize()
    return nc


def _core_inputs(x, w_attn, b_attn, w_proj):
    """Build the 8 per-core input maps (numpy float32)."""
    maps = []
    for core in range(NCORES):
        b = core // 4
        heads = [HPC * (core % 4) + k for k in range(HPC)]
        hA, hB, hC = heads
        qc = lambda h: slice(h * HS, (h + 1) * HS)
        kc = lambda h: slice(C + h * HS, C + (h + 1) * HS)
        vc = lambda h: slice(2 * C + h * HS, 2 * C + (h + 1) * HS)
        wqm = np.concatenate([
            w_attn[:, qc(hA)], w_attn[:, qc(hB)],
            w_attn[:, kc(hA)], w_attn[:, kc(hB)],
            w_attn[:, qc(hC)], w_attn[:, qc(hC)],
            w_attn[:, kc(hC)], w_attn[:, kc(hC)],
        ], axis=1)
        wvm = np.concatenate([w_attn[:, vc(h)] for h in heads], axis=1)
        bqm = np.zeros((128, 4), np.float32)
        bqm[0:64, 0] = b_attn[qc(hA)]
        bqm[64:128, 0] = b_attn[qc(hB)]
        bqm[0:64, 1] = b_attn[kc(hA)]
        bqm[64:128, 1] = b_attn[kc(hB)]
        bqm[0:64, 2] = b_attn[qc(hC)]
        bqm[64:128, 2] = b_attn[qc(hC)]
        bqm[0:64, 3] = b_attn[kc(hC)]
        bqm[64:128, 3] = b_attn[kc(hC)]
        wpabm = np.concatenate([w_proj[hA * HS:(hA + 1) * HS, :],
                                w_proj[hB * HS:(hB + 1) * HS, :]], axis=0)
        wpcm = w_proj[hC * HS:(hC + 1) * HS, :]
        import ml_dtypes
        bf = ml_dtypes.bfloat16
        maps.append({
            "xt": np.ascontiguousarray(x[b].T).astype(bf),
            "wq": np.ascontiguousarray(wqm).astype(bf),
            "wv": np.ascontiguousarray(wvm).astype(bf),
            "bq": np.ascontiguousarray(bqm, np.float32),
            "wpab": np.ascontiguousarray(wpabm).astype(bf),
            "wpc": np.ascontiguousarray(wpcm).astype(bf),
        })
    return maps


def run_cores(in_maps, trace=False):
    from concourse import bass_utils
    if "nc" not in _CACHE:
        _CACHE["nc"] = _build()
    return bass_utils.run_bass_kernel_spmd(
        _CACHE["nc"], in_maps, list(range(NCORES)), trace=trace)


def kernel(x, w_attn, b_attn, w_proj, b_proj):
    x = np.asarray(x, np.float32)
    w_attn = np.asarray(w_attn, np.float32)
    b_attn = np.asarray(b_attn, np.float32)
    w_proj = np.asarray(w_proj, np.float32)
    b_proj = np.asarray(b_proj, np.float32)

    # V-bias folds exactly into a constant row: sum_k P/den = 1, so
    # O_h = AV_h/den + bv_h and its projection adds bv_h @ W_h.
    b_eff = b_proj.astype(np.float64).copy()
    for h in range(NH):
        bv = b_attn[2 * C + h * HS:2 * C + (h + 1) * HS].astype(np.float64)
        b_eff += bv @ w_proj[h * HS:(h + 1) * HS, :].astype(np.float64)

    in_maps = _core_inputs(x, w_attn, b_attn, w_proj)
    res = run_cores(in_maps)
    y = np.zeros((B, T, C), np.float32)
    for b in range(B):
        acc = np.zeros((C, T), np.float64)
        for core in range(4 * b, 4 * b + 4):
            acc += res.results[core]["y"].astype(np.float64)
        y[b] = (acc.T + b_eff[None, :]).astype(np.float32)
    return y


# revision 14
# speedup vs baseline: 1.6199x; 1.0057x over previous
"""Causal self-attention kernel for 8 TRN2 NeuronCores (v3).

Problem (hardcoded): B=2, T=4096, C=768, NH=12, HS=64.
  qkv = x @ w_attn + b_attn; per-head causal softmax attention;
  y = att_out @ w_proj + b_proj

Sharding: 24 (batch, head) units over 8 cores -> 3 heads per core.
  cores 0..3: batch 0, heads (0,1,2), (3,4,5), (6,7,8), (9,10,11)
  cores 4..7: batch 1, same head split.
Each core computes a partial y^T [C, T]; the host sums partials per batch
and adds b_eff = b_proj + sum_h bv_h @ w_proj[h] (the V-bias contribution
commutes with the softmax average exactly: sum_k P/den = 1).

v3 structure (ScalarE exp is the hard floor at ~1 elem/cycle/lane;
everything else is arranged to keep ACT saturated and PE warm):
  - The QKV matmul groups for block tb+1 are emitted INTERLEAVED into the
    attention kt-loops of block tb, so the PE always has independent work
    during exp waits and ScalarE never drains at phase boundaries (v2
    showed ~22us ACT idle per tb and 14 HAM re-throttles without this).
  - Diagonal k-tiles are q-trimmed: S/exp/AV only touch q >= k_tile_base,
    the causal mask multiply shrinks to a constant [128,128] triangle.
  - V is computed directly in [token, dim] layout (stationary = x^T chunk,
    moving = wv) - no PE transposes (transpose-mode doesn't count as
    PE-busy for the HAM clock gate).
  - Normalize: den row staged to SBUF (custom-DVE ops mis-read PSUM on HW),
    reciprocal_approx_fast, gpsimd partition_broadcast, one DVE multiply.
  - Projection packs heads A,B into one 128-contraction matmul; head C's
    Q/K are duplicated ([qC|qC], [kC|kC]) so S_C pairs PE row groups.
"""

import numpy as np

B, T, C, NH = 2, 4096, 768, 12
HS = C // NH          # 64
NCORES = 8
HPC = 3               # heads per core
QB = 512              # q block (moving dim)
NQB = T // QB         # 8
NKT = T // 128        # 32 k-tiles
NTB = T // QB         # t-blocks
NCC = C // 128        # 6 contraction chunks
VP_W = 65             # per-head V block: 64 dims + ones column
WQJ = 4 * 128         # q/k weight blocks: [qA|qB | kA|kB | qC|qC | kC|kC]
WVJ = 192             # v moving width (3 heads x 64)

_CACHE = {}


def _build():
    import contextlib
    import concourse.bacc as bacc
    import concourse.mybir as mybir
    from concourse.tile import TileContext

    f32 = mybir.dt.float32
    f32r = mybir.dt.float32r
    bf16 = mybir.dt.bfloat16
    Exp = mybir.ActivationFunctionType.Exp
    mult = mybir.AluOpType.mult

    nc = bacc.Bacc(trn_type="TRN2")

    xt_d = nc.dram_tensor("xt", [C, T], bf16, kind="ExternalInput")
    wq = nc.dram_tensor("wq", [C, WQJ], bf16, kind="ExternalInput")
    wv = nc.dram_tensor("wv", [C, WVJ], bf16, kind="ExternalInput")
    bq = nc.dram_tensor("bq", [128, 4], f32, kind="ExternalInput")
    wpab = nc.dram_tensor("wpab", [128, C], bf16, kind="ExternalInput")
    wpc = nc.dram_tensor("wpc", [64, C], bf16, kind="ExternalInput")
    y = nc.dram_tensor("y", [C, T], f32, kind="ExternalOutput")

    with TileContext(nc) as tc, nc.allow_low_precision("f32r kernel"):
        with contextlib.ExitStack() as ctx:
            cpool = ctx.enter_context(tc.tile_pool(name="const", bufs=1))
            keep = ctx.enter_context(tc.tile_pool(name="keep", bufs=1))
            xtp_p = ctx.enter_context(tc.tile_pool(name="xtp", bufs=2))
            pt_p = ctx.enter_context(tc.tile_pool(name="ptp", bufs=6))
            on_p = ctx.enter_context(tc.tile_pool(name="onp", bufs=2))
            rr_p = ctx.enter_context(tc.tile_pool(name="rrp", bufs=2))
            ys_p = ctx.enter_context(tc.tile_pool(name="ysp", bufs=2))
            sps_p = ctx.enter_context(
                tc.tile_pool(name="sps", bufs=2, space="PSUM"))
            ov_p = ctx.enter_context(
                tc.tile_pool(name="ovp", bufs=1, space="PSUM"))
            sm_p = ctx.enter_context(
                tc.tile_pool(name="smp", bufs=2, space="PSUM"))

            # constant [128,128] lower-triangle mask: tri[k, q] = (k <= q)
            tri = cpool.tile([128, 128], f32)
            nc.gpsimd.memset(tri[:], 1.0)
            nc.gpsimd.affine_select(
                out=tri[:], in_=tri[:], compare_op=mybir.AluOpType.is_ge,
                fill=0.0, base=0, channel_multiplier=-1, pattern=[[1, 128]])
            tri_b = cpool.tile([128, 128], bf16)
            nc.vector.tensor_copy(tri_b[:], tri[:])

            wq_sb = cpool.tile([128, NCC, WQJ], bf16)
            for cc in range(NCC):
                nc.gpsimd.dma_start(wq_sb[:, cc, :],
                                    wq[cc * 128:(cc + 1) * 128, :])
            wv_sb = cpool.tile([128, NCC, WVJ], bf16)
            nc.sync.dma_start(wv_sb[:],
                              wv.rearrange("(cc p) j -> p cc j", p=128))
            bq_sb = cpool.tile([128, 4], f32)
            nc.sync.dma_start(bq_sb[:], bq[:, :])
            wpab_sb = cpool.tile([128, C], bf16)
            wpc_sb = cpool.tile([64, C], bf16)

            warm_ps = sm_p.tile([128, 128], f32, tag="small", name="warm")
            for i in range(12):
                nc.tensor.matmul(warm_ps[:], tri[:], tri[:],
                                 start=True, stop=True)
            bc_dum = cpool.tile([64, 4], f32)
            nc.gpsimd.partition_broadcast(bc_dum[:], bq_sb[0:1, 0:4])

            QT_AB = keep.tile([128, T], bf16, tag="qt_ab")
            KT_AB = keep.tile([128, T], bf16, tag="kt_ab")
            QT_C = keep.tile([128, T], bf16, tag="qt_c")
            KT_C = keep.tile([128, T], bf16, tag="kt_c")
            V3 = keep.tile([128, NKT, HPC, VP_W], bf16, tag="v3")
            # persistent ones column at [..., 64]; V copies only write 0:64
            nc.gpsimd.memset(V3[:, :, :, 64:65], 1.0)

            QK_DEST = [QT_AB, KT_AB, QT_C, KT_C]

            _xt = {}

            def qkv_dmas(tb):
                t0 = tb * QB
                xt = xtp_p.tile([128, NCC, QB], bf16, tag="xt",
                                name=f"xt{tb}")
                eng = nc.sync if tb == 0 else nc.gpsimd
                for cc in range(NCC):
                    eng.dma_start(
                        xt[:, cc, :],
                        xt_d[cc * 128:(cc + 1) * 128, t0:t0 + QB])
                _xt[tb] = xt

            def qkv_group(tb, g):
                """g in 0..3: Q/K weight block g; g in 4..7: V token tile."""
                t0 = tb * QB
                xt = _xt[tb]
                if g < 4:
                    qp = sm_p.tile([128, QB], f32, tag="small",
                                   name=f"qp{tb}_{g}")
                    for cc in range(NCC):
                        nc.tensor.matmul(
                            qp[:], wq_sb[:, cc, g * 128:(g + 1) * 128],
                            xt[:, cc, :],
                            start=(cc == 0), stop=(cc == NCC - 1))
                    nc.vector.tensor_scalar_add(
                        QK_DEST[g][:, t0:t0 + QB], qp[:], bq_sb[:, g:g + 1])
                else:
                    i = g - 4
                    vp = sm_p.tile([128, WVJ], f32, tag="small",
                                   name=f"vp{tb}_{i}")
                    for cc in range(NCC):
                        nc.tensor.matmul(
                            vp[:], xt[:, cc, i * 128:(i + 1) * 128],
                            wv_sb[:, cc, :],
                            start=(cc == 0), stop=(cc == NCC - 1))
                    dst = V3[:, tb * 4 + i, :, 0:64]
                    src = vp[:, :].rearrange("p (h d) -> p h d", h=HPC)
                    nc.vector.tensor_copy(dst, src)

            def normalize(ov, dest, nm):
                # one copy stages ov to SBUF and frees its PSUM bank; the
                # custom-DVE recip needs an SBUF source on HW anyway
                osb = rr_p.tile([65, QB], f32, tag="osb", name=f"os{nm}")
                nc.vector.tensor_copy(osb[:], ov[:])
                # the custom-DVE recip needs a partition-0 SBUF source on HW
                dsb = rr_p.tile([1, QB], f32, tag="dsb", name=f"ds{nm}")
                nc.vector.tensor_copy(dsb[:], ov[64:65, :])
                rr = rr_p.tile([1, QB], f32, tag="rr", name=f"rr{nm}")
                nc.vector.reciprocal_approx_fast(rr[:], dsb[:])
                rrb = rr_p.tile([64, QB], f32, tag="rrb", name=f"rrb{nm}")
                nc.gpsimd.partition_broadcast(rrb[:], rr[:])
                nc.vector.tensor_tensor(out=dest, in0=osb[0:64, :],
                                        in1=rrb[:], op=mult)

            def proj_unit(qb, co, onAB, onC):
                q0 = qb * QB
                yp = sm_p.tile([128, QB], f32, tag="small",
                               name=f"yp{qb}_{co}")
                nc.tensor.matmul(
                    yp[:], wpab_sb[:, co * 128:(co + 1) * 128],
                    onAB[:], start=True, stop=False)
                nc.tensor.matmul(
                    yp[:], wpc_sb[:, co * 128:(co + 1) * 128],
                    onC[:], start=False, stop=True)
                ys = ys_p.tile([128, QB], f32, tag="ys",
                               name=f"ys{qb}_{co}")
                nc.vector.tensor_copy(ys[:], yp[:])
                nc.sync.dma_start(
                    y[co * 128:(co + 1) * 128, q0:q0 + QB], ys[:])

            qkv_dmas(0)
            # remaining proj weights can load behind the first x/wq chunks
            nc.gpsimd.dma_start(wpab_sb[:], wpab[:, :])
            nc.gpsimd.dma_start(wpc_sb[:], wpc[:, :])
            qkv_group(0, 0)
            qkv_group(0, 1)
            prev_on = None  # (qb, onAB, onC) awaiting projection

            for tb in range(NTB):
                qb = tb
                q0 = qb * QB
                nkt = 4 * qb + 4

                # filler units spread across this block's attention
                # iterations: prev block's projection (ready immediately),
                # tb=0's own remaining QKV groups, next block's QKV groups
                units = []
                if prev_on is not None:
                    pq, pAB, pC = prev_on
                    for co in range(NCC):
                        units.append(("proj", pq, co, pAB, pC))
                    prev_on = None
                if tb + 1 < NTB:
                    qkv_dmas(tb + 1)
                    for g in range(8):
                        units.append(("qkv", tb + 1, g))
                filler = {}
                total = nkt + nkt // 2
                nun = max(1, len(units))
                for u, unit in enumerate(units):
                    pos = min(total - 1, (u * total) // nun)
                    filler.setdefault(pos, []).append(unit)

                def run_filler(pos):
                    for unit in filler.get(pos, ()):
                        if unit[0] == "qkv":
                            qkv_group(unit[1], unit[2])
                        else:
                            proj_unit(unit[1], unit[2], unit[3], unit[4])

                def pre_av(kt):
                    # tb=0 bootstrap: this kt's V tile right before its AV,
                    # C's Q/K blocks behind the first two iterations
                    if tb == 0:
                        qkv_group(0, 4 + kt)
                        if kt < 2:
                            qkv_group(0, 2 + kt)

                def ab_phase(fpos0):
                    # heads A, B: row-group-alternated S, shared exp
                    ovA = ov_p.tile([65, QB], f32, tag="ovA",
                                    name=f"ovA{qb}")
                    ovB = ov_p.tile([65, QB], f32, tag="ovB",
                                    name=f"ovB{qb}")
                    for kt in range(nkt):
                        k0 = kt * 128
                        m = k0 - q0
                        qo = max(0, m)  # q-trim on diagonal tiles
                        sps = sps_p.tile([128, 1024], f32, tag="sps",
                                         name=f"sAB{qb}_{kt}")
                        nc.tensor.matmul(
                            sps[:, qo:QB], KT_AB[0:64, k0:k0 + 128],
                            QT_AB[0:64, q0 + qo:q0 + QB],
                            start=True, stop=True)
                        nc.tensor.matmul(
                            sps[:, QB + qo:2 * QB],
                            KT_AB[64:128, k0:k0 + 128],
                            QT_AB[64:128, q0 + qo:q0 + QB],
                            start=True, stop=True)
                        pt = pt_p.tile([128, 2, QB], bf16, tag="pt")
                        spsv = sps[:].rearrange("p (h q) -> p h q", h=2)
                        nc.scalar.activation(pt[:, :, qo:QB],
                                             spsv[:, :, qo:QB],
                                             Exp, scale=0.125)
                        if 0 <= m < QB:
                            for h in range(2):
                                nc.vector.tensor_tensor(
                                    out=pt[:, h, m:m + 128],
                                    in0=pt[:, h, m:m + 128],
                                    in1=tri_b[:], op=mult)
                        pre_av(kt)
                        nc.tensor.matmul(ovA[:, qo:QB], V3[:, kt, 0, :],
                                         pt[:, 0, qo:QB],
                                         start=(kt == 0),
                                         stop=(kt == nkt - 1))
                        nc.tensor.matmul(ovB[:, qo:QB], V3[:, kt, 1, :],
                                         pt[:, 1, qo:QB],
                                         start=(kt == 0),
                                         stop=(kt == nkt - 1))
                        run_filler(fpos0 + kt)
                    onAB = on_p.tile([128, QB], bf16, tag="onAB",
                                     name=f"onAB{qb}")
                    normalize(ovA, onAB[0:64, :], f"A{qb}")
                    normalize(ovB, onAB[64:128, :], f"B{qb}")
                    return onAB

                def c_phase(fpos0):
                    # head C: row alternation via duplicated Q/K rows
                    # (even kt on rows 0-63, odd kt on rows 64-127)
                    ovC = ov_p.tile([65, QB], f32, tag="ovA",
                                    name=f"ovC{qb}")
                    for s in range(nkt // 2):
                        kt0, kt1 = 2 * s, 2 * s + 1
                        m0 = kt0 * 128 - q0
                        m1 = m0 + 128
                        qo0, qo1 = max(0, m0), max(0, m1)
                        sps = sps_p.tile([128, 1024], f32, tag="sps",
                                         name=f"sC{qb}_{s}")
                        nc.tensor.matmul(
                            sps[:, qo0:QB],
                            KT_C[0:64, kt0 * 128:kt0 * 128 + 128],
                            QT_C[0:64, q0 + qo0:q0 + QB],
                            start=True, stop=True)
                        # half1 writes the full shared range [qo0, QB) so
                        # the shared exp reads only written PSUM; its AV
                        # reads [qo1, QB)
                        nc.tensor.matmul(
                            sps[:, QB + qo0:2 * QB],
                            KT_C[64:128, kt1 * 128:kt1 * 128 + 128],
                            QT_C[64:128, q0 + qo0:q0 + QB],
                            start=True, stop=True)
                        pt = pt_p.tile([128, 2, QB], bf16, tag="pt")
                        spsv = sps[:].rearrange("p (h q) -> p h q", h=2)
                        nc.scalar.activation(pt[:, :, qo0:QB],
                                             spsv[:, :, qo0:QB],
                                             Exp, scale=0.125)
                        for half, m in ((0, m0), (1, m1)):
                            if 0 <= m < QB:
                                nc.vector.tensor_tensor(
                                    out=pt[:, half, m:m + 128],
                                    in0=pt[:, half, m:m + 128],
                                    in1=tri_b[:], op=mult)
                        nc.tensor.matmul(ovC[:, qo0:QB], V3[:, kt0, 2, :],
                                         pt[:, 0, qo0:QB],
                                         start=(s == 0), stop=False)
                        nc.tensor.matmul(ovC[:, qo1:QB], V3[:, kt1, 2, :],
                                         pt[:, 1, qo1:QB],
                                         start=False,
                                         stop=(s == nkt // 2 - 1))
                        run_filler(fpos0 + s)
                    onC = on_p.tile([64, QB], bf16, tag="onC",
                                    name=f"onC{qb}")
                    normalize(ovC, onC[:], f"C{qb}")
                    return onC

                if tb < NTB - 1:
                    onAB = ab_phase(0)
                    onC = c_phase(nkt)
                else:
                    # last block: C first, so the final normalize/gpsimd
                    # drain overlaps the AB loop and projection
                    onC = c_phase(0)
                    onAB = ab_phase(nkt // 2)
                # projection deferred: runs as filler inside the next
                # block's attention loops (keeps ACT fed at the boundary)
                prev_on = (qb, onAB, onC)

            pq, pAB, pC = prev_on
            for co in range(NCC):
                proj_unit(pq, co, pAB, pC)

    nc.finalize()
    return nc


def _core_inputs(x, w_attn, b_attn, w_proj):
    """Build the 8 per-core input maps (numpy float32)."""
    maps = []
    for core in range(NCORES):
        b = core // 4
        heads = [HPC * (core % 4) + k for k in range(HPC)]
        hA, hB, hC = heads
        qc = lambda h: slice(h * HS, (h + 1) * HS)
        kc = lambda h: slice(C + h * HS, C + (h + 1) * HS)
        vc = lambda h: slice(2 * C + h * HS, 2 * C + (h + 1) * HS)
        wqm = np.concatenate([
            w_attn[:, qc(hA)], w_attn[:, qc(hB)],
            w_attn[:, kc(hA)], w_attn[:, kc(hB)],
            w_attn[:, qc(hC)], w_attn[:, qc(hC)],
            w_attn[:, kc(hC)], w_attn[:, kc(hC)],
        ], axis=1)
        wvm = np.concatenate([w_attn[:, vc(h)] for h in heads], axis=1)
        bqm = np.zeros((128, 4), np.float32)
        bqm[0:64, 0] = b_attn[qc(hA)]
        bqm[64:128, 0] = b_attn[qc(hB)]
        bqm[0:64, 1] = b_attn[kc(hA)]
        bqm[64:128, 1] = b_attn[kc(hB)]
        bqm[0:64, 2] = b_attn[qc(hC)]
        bqm[64:128, 2] = b_attn[qc(hC)]
        bqm[0:64, 3] = b_attn[kc(hC)]
        bqm[64:128, 3] = b_attn[kc(hC)]
        wpabm = np.concatenate([w_proj[hA * HS:(hA + 1) * HS, :],
                                w_proj[hB * HS:(hB + 1) * HS, :]], axis=0)
        wpcm = w_proj[hC * HS:(hC + 1) * HS, :]
        import ml_dtypes
        bf = ml_dtypes.bfloat16
        maps.append({
            "xt": np.ascontiguousarray(x[b].T).astype(bf),
            "wq": np.ascontiguousarray(wqm).astype(bf),
            "wv": np.ascontiguousarray(wvm).astype(bf),
            "bq": np.ascontiguousarray(bqm, np.float32),
            "wpab": np.ascontiguousarray(wpabm).astype(bf),
            "wpc": np.ascontiguousarray(wpcm).astype(bf),
        })
    return maps


def run_cores(in_maps, trace=False):
    from concourse import bass_utils
    if "nc" not in _CACHE:
        _CACHE["nc"] = _build()
    return bass_utils.run_bass_kernel_spmd(
        _CACHE["nc"], in_maps, list(range(NCORES)), trace=trace)


def kernel(x, w_attn, b_attn, w_proj, b_proj):
    x = np.asarray(x, np.float32)
    w_attn = np.asarray(w_attn, np.float32)
    b_attn = np.asarray(b_attn, np.float32)
    w_proj = np.asarray(w_proj, np.float32)
    b_proj = np.asarray(b_proj, np.float32)

    # V-bias folds exactly into a constant row: sum_k P/den = 1, so
    # O_h = AV_h/den + bv_h and its projection adds bv_h @ W_h.
    b_eff = b_proj.astype(np.float64).copy()
    for h in range(NH):
        bv = b_attn[2 * C + h * HS:2 * C + (h + 1) * HS].astype(np.float64)
        b_eff += bv @ w_proj[h * HS:(h + 1) * HS, :].astype(np.float64)

    in_maps = _core_inputs(x, w_attn, b_attn, w_proj)
    res = run_cores(in_maps)
    y = np.zeros((B, T, C), np.float32)
    for b in range(B):
        acc = np.zeros((C, T), np.float64)
        for core in range(4 * b, 4 * b + 4):
            acc += res.results[core]["y"].astype(np.float64)
        y[b] = (acc.T + b_eff[None, :]).astype(np.float32)
    return y
